# revision 44
# baseline (speedup 1.0000x reference)
"""Trainium2 Bass kernel for nn_CBlock2 (sparse cluster attention block).

Strategy: data-parallel over batch B=8 across 8 cores. Per core, tokens are
host-sorted by cluster id so same-cluster attention pairs lie in a 3-tile
band around the diagonal; the cluster mask is folded into the score matmul
via one-hot augmentation rows (exp(s - 50) == 0 for cross-cluster pairs).
LayerNorm gains/biases are folded into the adjacent weight matrices on the
host. All matmuls run in bf16 with fp32 PSUM accumulation; the residual
stream stays fp32.
"""
import sys

sys.path.insert(0, "/opt/trn_rl_repo")

import numpy as np
import ml_dtypes

import concourse.bass as bass
import concourse.mybir as mybir
import concourse.tile as tile
from concourse.bass_utils import run_bass_kernel_spmd
from concourse.masks import make_identity

BF16 = ml_dtypes.bfloat16

B, N, C, H, PD, CLN = 8, 1024, 512, 8, 256, 64
HD = C // H          # 64
HID = 4 * C          # 2048
LN_EPS = 1e-5
ATT_EPS = 1e-6
P = 128
NT = N // P          # 8 token tiles
KC = C // P          # 4 contraction tiles over C
FH = HID // P        # 16 hidden tiles
SCALE = HD ** -0.5   # 0.125
ALPHA_Q = 16.0
ALPHA_K = 25.0
BIG = ALPHA_Q * ALPHA_K * SCALE  # 50.0: mask margin inside exp
MAX_CLUSTER = 65     # band bound: all same-cluster pairs within +-64 tokens

F32 = mybir.dt.float32
BF = mybir.dt.bfloat16


def _split_excess_waits(nc, max_waits=1):
    """walrus in this env rejects >1 sync-wait on one instruction; hoist
    excess waits onto same-engine no-op carriers inserted just before."""
    for f in nc.m.functions:
        for bb in f.blocks:
            new_insts = []
            for inst in bb.instructions:
                si = inst.sync_info
                if si is not None and si.on_wait and len(si.on_wait) > max_waits:
                    waits = list(si.on_wait)
                    excess, keep = waits[:-max_waits], waits[-max_waits:]
                    for ci in range(0, len(excess), max_waits):
                        chunk = excess[ci : ci + max_waits]
                        new_insts.append(
                            mybir.InstNoOp(
                                name=f"{inst.name}-ws{ci}",
                                engine=inst.engine,
                                ins=[],
                                outs=[],
                                sync_info=mybir.SyncInfo(on_wait=chunk, on_update=[]),
                            )
                        )
                    inst.sync_info = mybir.SyncInfo(
                        on_wait=keep, on_update=list(si.on_update)
                    )
                new_insts.append(inst)
            bb.instructions = new_insts


_PROGRAM = None


def _band_range(jt):
    """i-range covered by the score tile of j-tile jt (+-64-token band)."""
    i0 = max(0, jt * P - 64)
    i1 = min(N, (jt + 1) * P + 64)
    return i0, i1


def _build_program(with_biases: bool):
    """Build the per-core SPMD program. with_biases=False omits the bias
    paths (all reference biases are zero in the standard setup)."""
    nc = bass.Bass()

    x_d = nc.declare_dram_parameter("x", [N, C], F32, isOutput=False)
    wqT_d = nc.declare_dram_parameter("wqT", [C, C], BF, isOutput=False)
    wkT_d = nc.declare_dram_parameter("wkT", [C, C], BF, isOutput=False)
    wvT_d = nc.declare_dram_parameter("wvT", [C, C], BF, isOutput=False)
    wpT_d = nc.declare_dram_parameter("wpT", [C, C], BF, isOutput=False)
    w1T_d = nc.declare_dram_parameter("w1T", [C, HID], BF, isOutput=False)
    w2T_d = nc.declare_dram_parameter("w2T", [HID, C], BF, isOutput=False)
    m1b_d = nc.declare_dram_parameter("m1b", [HID], F32, isOutput=False)
    qm_d = nc.declare_dram_parameter("qmask", [CLN, N], BF, isOutput=False)
    km_d = nc.declare_dram_parameter("kmask", [CLN, N], BF, isOutput=False)
    if with_biases:
        qb_d = nc.declare_dram_parameter("qb", [C], F32, isOutput=False)
        kb_d = nc.declare_dram_parameter("kb", [C], F32, isOutput=False)
        vb_d = nc.declare_dram_parameter("vb", [C], BF, isOutput=False)
        pb_d = nc.declare_dram_parameter("pb", [C], BF, isOutput=False)
        m2b_d = nc.declare_dram_parameter("m2b", [C], BF, isOutput=False)
    y_d = nc.declare_dram_parameter("y", [N, C], F32, isOutput=True)

    with tile.TileContext(nc) as tc:
        from contextlib import ExitStack

        with ExitStack() as ctx:
            ec = ctx.enter_context
            persist = ec(tc.tile_pool(name="persist", bufs=1))
            w_small = ec(tc.tile_pool(name="w_small", bufs=1))
            w_big = ec(tc.tile_pool(name="w_big", bufs=1))
            xs_pool = ec(tc.tile_pool(name="xs", bufs=4))
            xr_pool = ec(tc.tile_pool(name="xr", bufs=2))
            ln_pool = ec(tc.tile_pool(name="ln", bufs=6))
            e_pool = ec(tc.tile_pool(name="epool", bufs=6))
            r_pool = ec(tc.tile_pool(name="rpool", bufs=8))
            out_pool = ec(tc.tile_pool(name="outp", bufs=3))

            # ---- tiny constants (cheap, emitted first) ----
            ident = persist.tile([P, P], BF, tag="ident")
            make_identity(nc, ident[:])
            eps_t = persist.tile([P, 1], F32, tag="eps_t")
            nc.vector.memset(eps_t[:], LN_EPS)
            nbig_t = persist.tile([P, 1], F32, tag="nbig_t")
            nc.vector.memset(nbig_t[:], -BIG)
            if with_biases:
                ones_row = persist.tile([1, P], BF, tag="ones_row")
                nc.vector.memset(ones_row[:], 1.0)

            # ---- persistent activations ----
            # qA/kA: one wide tile, head h at free cols [h*N,(h+1)*N). Even
            # heads keep q rows on partitions 0:64 + mask rows on 64:128;
            # odd heads are flipped so PSUM->SBUF copies stay
            # partition-aligned.
            qA = persist.tile([P, H * N], BF, tag="qA")
            kA = persist.tile([P, H * N], BF, tag="kA")
            vext = [persist.tile([P, H, HD + 1], BF, tag=f"vx{t}", name=f"vx{t}") for t in range(NT)]
            uT = persist.tile([P, KC, N], BF, tag="uT")
            OT = persist.tile([P, KC, N], BF, tag="OT")
            O_sb = [persist.tile([P, C], BF, tag=f"Osb{t}", name=f"Osb{t}") for t in range(NT)]
            x1 = [persist.tile([P, C], F32, tag=f"x1{t}", name=f"x1{t}") for t in range(NT)]
            h1T = [persist.tile([P, N], BF, tag=f"h1T{k}", name=f"h1T{k}") for k in range(FH)]

            wq_t = [w_small.tile([P, C], BF, tag=f"wq{k}", name=f"wq{k}") for k in range(KC)]
            wk_t = [w_small.tile([P, C], BF, tag=f"wk{k}", name=f"wk{k}") for k in range(KC)]
            wv_t = [w_small.tile([P, C], BF, tag=f"wv{k}", name=f"wv{k}") for k in range(KC)]
            wp_t = [w_small.tile([P, C], BF, tag=f"wp{k}", name=f"wp{k}") for k in range(KC)]
            w2_t = [w_small.tile([P, C], BF, tag=f"w2{k}", name=f"w2{k}") for k in range(FH)]
            w1_t = [w_big.tile([P, HID], BF, tag=f"w1{k}", name=f"w1{k}") for k in range(KC)]

            def layernorm_to_uT(src_tile_fn, dst_uT, ps_m):
                """token-major f32 tiles -> normalized bf16, PE-transposed into
                feature-major dst_uT (KC tiles of [P, N])."""
                for it in range(NT):
                    xt = src_tile_fn(it)
                    stats = ln_pool.tile([P, 6], F32, tag="stats")
                    nc.vector.bn_stats(out=stats[:], in_=xt[:])
                    mv = ln_pool.tile([P, 2], F32, tag="mv")
                    nc.vector.bn_aggr(out=mv[:], in_=stats[:])
                    std = ln_pool.tile([P, 1], F32, tag="std")
                    nc.scalar.activation(
                        out=std[:], in_=mv[:, 1:2],
                        func=mybir.ActivationFunctionType.Sqrt,
                        bias=eps_t[:], scale=1.0,
                    )
                    nc.vector.reciprocal(out=std[:], in_=std[:])
                    u = ln_pool.tile([P, C], BF, tag="u")
                    nc.vector.tensor_scalar(
                        out=u[:], in0=xt[:],
                        scalar1=mv[:, 0:1], scalar2=std[:],
                        op0=mybir.AluOpType.subtract, op1=mybir.AluOpType.mult,
                    )
                    pt = ps_m.tile([P, KC, P], BF, tag="misc")
                    for k in range(KC):
                        nc.tensor.transpose(
                            pt[:, k, :], u[:, k * P : (k + 1) * P], ident[:]
                        )
                    nc.scalar.copy(
                        out=dst_uT[:, :, it * P : (it + 1) * P], in_=pt[:]
                    )

            # ---- phase 1: LN1 (x streamed in first -- nothing queues ahead) ----
            def _x_src(it):
                xt = xs_pool.tile([P, C], F32, tag="xt")
                nc.sync.dma_start(out=xt[:], in_=x_d[it * P : (it + 1) * P, :])
                return xt

            with nc.named_scope("ln1"), \
                    tc.tile_pool(name="ps_m1", bufs=2, space="PSUM") as ps_m1:
                layernorm_to_uT(_x_src, uT, ps_m1)

            # qkv weights arrive while LN1 runs
            for k in range(KC):
                nc.sync.dma_start(out=wq_t[k][:], in_=wqT_d[k * P : (k + 1) * P, :])
            for k in range(KC):
                nc.sync.dma_start(out=wk_t[k][:], in_=wkT_d[k * P : (k + 1) * P, :])
            for k in range(KC):
                nc.sync.dma_start(out=wv_t[k][:], in_=wvT_d[k * P : (k + 1) * P, :])
            for mk, dstA in ((qm_d, qA), (km_d, kA)):
                map_ = mk[:]
                src_b = bass.AP(
                    tensor=map_.tensor, offset=map_.offset,
                    ap=[map_.ap[0], [0, H // 2], map_.ap[1]],
                )
                ev = dstA[:].rearrange("p (h n) -> p h n", h=H)
                nc.sync.dma_start(out=ev[HD:P, 0:H:2, :], in_=src_b)
                nc.sync.dma_start(out=ev[0:HD, 1:H:2, :], in_=src_b)
            m1b_t = persist.tile([P, FH], F32, tag="m1b")
            nc.sync.dma_start(out=m1b_t[:], in_=m1b_d.rearrange("(f p) -> p f", p=P))
            if with_biases:
                qb_t = persist.tile([P, KC], F32, tag="qb")
                nc.sync.dma_start(
                    out=qb_t[:], in_=qb_d.rearrange("(c p) -> p c", p=P)
                )
                kb_t = persist.tile([P, KC], F32, tag="kb")
                nc.sync.dma_start(
                    out=kb_t[:], in_=kb_d.rearrange("(c p) -> p c", p=P)
                )
                vb_r = persist.tile([1, C], BF, tag="vb_r")
                nc.sync.dma_start(out=vb_r[:], in_=vb_d.rearrange("(a c) -> a c", a=1))
                pb_r = persist.tile([1, C], BF, tag="pb_r")
                nc.sync.dma_start(out=pb_r[:], in_=pb_d.rearrange("(a c) -> a c", a=1))
                m2b_r = persist.tile([1, C], BF, tag="m2b_r")
                nc.sync.dma_start(
                    out=m2b_r[:], in_=m2b_d.rearrange("(a c) -> a c", a=1)
                )

            # ---- phase 2: QKV ----
            # q/k feature-major into augmented head tiles (partitions 0:64):
            # q copies on ACT, k copies on DVE to balance engine load.
            with nc.named_scope("qkv"), \
                    tc.tile_pool(name="ps_qkv", bufs=6, space="PSUM") as ps_acc:
                for which, w_t, dstA in (("q", wq_t, qA), ("k", wk_t, kA)):
                    for co in range(KC):  # output-feature tile: heads 2co, 2co+1
                        for nch in range(2):  # 512-token chunks
                            ps = ps_acc.tile([P, C], F32, tag="acc")
                            for k in range(KC):
                                nc.tensor.matmul(
                                    ps[:],
                                    w_t[k][:, co * P : (co + 1) * P],
                                    uT[:, k, nch * 512 : (nch + 1) * 512],
                                    start=(k == 0), stop=(k == KC - 1),
                                )
                            for half in range(2):
                                h = 2 * co + half
                                pr = slice(half * HD, (half + 1) * HD)
                                dst = dstA[pr, h * N + nch * 512 : h * N + (nch + 1) * 512]
                                src = ps[pr, :]
                                if which == "q":
                                    if with_biases:
                                        nc.scalar.activation(
                                            out=dst, in_=src,
                                            func=mybir.ActivationFunctionType.Identity,
                                            bias=qb_t[half * HD : (half + 1) * HD, co : co + 1],
                                            scale=1.0,
                                        )
                                    else:
                                        nc.scalar.copy(out=dst, in_=src)
                                else:
                                    if with_biases:
                                        nc.vector.tensor_scalar_add(
                                            dst, src,
                                            kb_t[half * HD : (half + 1) * HD, co : co + 1],
                                        )
                                    elif (co + nch) % 2 == 0:
                                        nc.vector.tensor_copy(out=dst, in_=src)
                                    else:
                                        nc.scalar.copy(out=dst, in_=src)
                # v token-major packed into vext (+ ones column for denominators)
                for it in range(NT):
                    ps = ps_acc.tile([P, C], F32, tag="acc")
                    for k in range(KC):
                        nc.tensor.matmul(
                            ps[:],
                            uT[:, k, it * P : (it + 1) * P],
                            wv_t[k][:],
                            start=(k == 0),
                            stop=(k == KC - 1) and not with_biases,
                        )
                    if with_biases:
                        nc.tensor.matmul(
                            ps[:], ones_row[:], vb_r[:], start=False, stop=True
                        )
                    vx = vext[it]
                    nc.gpsimd.memset(vx[:, :, HD : HD + 1], 1.0)
                    nc.scalar.copy(
                        out=vx[:, :, 0:HD],
                        in_=ps[:].rearrange("p (h d) -> p h d", h=H),
                    )

            # proj weights arrive during QKV/attention
            for k in range(KC):
                nc.sync.dma_start(out=wp_t[k][:], in_=wpT_d[k * P : (k + 1) * P, :])

            # ---- phase 3+4: per-head banded scores/exp then AV ----
            # score tiles for j-tile pairs (2jp, 2jp+1) share one PSUM tile:
            # pair widths are <=512 so both sub-tiles fit one PSUM bank and
            # exp runs as one wide activation per pair.
            pair_off = {}
            for jt in range(NT):
                i0, i1 = _band_range(jt)
                pair_off[jt] = 0 if jt % 2 == 0 else (
                    _band_range(jt - 1)[1] - _band_range(jt - 1)[0]
                )
            with nc.named_scope("attn"), \
                    tc.tile_pool(name="ps_sc", bufs=2, space="PSUM") as ps_s, \
                    tc.tile_pool(name="ps_po", bufs=4, space="PSUM") as ps_m:
                # quad offsets: jts 4g..4g+3 share one 2-bank psum tile; subs
                # 0,1 pack into bank0 and 2,3 into bank1 (a matmul output
                # must stay within one PSUM bank).
                qoff = {}
                for jt in range(NT):
                    g, sub = jt // 4, jt % 4
                    if sub == 0:
                        qoff[jt] = 0
                    elif sub == 1:
                        qoff[jt] = _band_range(jt - 1)[1] - _band_range(jt - 1)[0]
                    elif sub == 2:
                        qoff[jt] = 512
                    else:
                        qoff[jt] = 512 + _band_range(jt - 1)[1] - _band_range(jt - 1)[0]
                for h in range(H):
                    e_tiles = {}
                    for g in range(NT // 4):
                        jlast = 4 * g + 3
                        wtot = qoff[jlast] + _band_range(jlast)[1] - _band_range(jlast)[0]
                        ps = ps_s.tile([P, 1024], F32, tag="sc")
                        for sub in range(4):
                            jt = 4 * g + sub
                            i0, i1 = _band_range(jt)
                            nc.tensor.matmul(
                                ps[:, qoff[jt] : qoff[jt] + (i1 - i0)],
                                kA[:, h * N + jt * P : h * N + (jt + 1) * P],
                                qA[:, h * N + i0 : h * N + i1],
                                start=True, stop=True,
                            )
                        et = e_pool.tile([P, 1024], BF, tag="et")
                        nc.scalar.activation(
                            out=et[:, 0:wtot], in_=ps[:, 0:wtot],
                            func=mybir.ActivationFunctionType.Exp,
                            bias=nbig_t[:], scale=SCALE,
                        )
                        e_tiles[g] = et
                    for it in range(NT):
                        po = ps_m.tile([P, HD + 1], F32, tag="misc")
                        # center tile zero-initializes the whole psum region;
                        # the +-64 edge pieces accumulate partial partitions.
                        c_off = qoff[it] + it * P - _band_range(it)[0]
                        nc.tensor.matmul(
                            po[:],
                            e_tiles[it // 4][:, c_off : c_off + P],
                            vext[it][:, h, :],
                            start=True, stop=False,
                        )
                        pieces = []
                        if it > 0:
                            jt = it - 1
                            off = qoff[jt] + it * P - _band_range(jt)[0]
                            pieces.append((jt, off, 0))      # po[0:64]
                        if it < NT - 1:
                            jt = it + 1
                            off = qoff[jt]                   # i starts at its i0
                            pieces.append((jt, off, HD))     # po[64:128]
                        for n_, (jt, off, pbase) in enumerate(pieces):
                            nc.tensor.matmul(
                                po[pbase : pbase + HD, :],
                                e_tiles[jt // 4][:, off : off + HD],
                                vext[jt][:, h, :],
                                start=False, stop=(n_ == len(pieces) - 1),
                            )
                        # D >= ~0.4 always, so the reference's +1e-6 in the
                        # denominator is far below bf16 noise; skip it.
                        r = r_pool.tile([P, 1], F32, tag="r")
                        nc.vector.reciprocal(r[:], po[:, HD : HD + 1])
                        nc.vector.tensor_scalar_mul(
                            O_sb[it][:, h * HD : (h + 1) * HD], po[:, 0:HD], r[:]
                        )

            # mlp weights arrive during attention
            for k in range(KC):
                nc.sync.dma_start(out=w1_t[k][:], in_=w1T_d[k * P : (k + 1) * P, :])
            for k in range(FH):
                nc.sync.dma_start(out=w2_t[k][:], in_=w2T_d[k * P : (k + 1) * P, :])

            # ---- phase 5: transpose O + proj + residual ----
            ps_m2 = ec(tc.tile_pool(name="ps_m2", bufs=2, space="PSUM"))
            ps_acc = ec(tc.tile_pool(name="ps_acc2", bufs=5, space="PSUM"))
            ps_m = ps_m2
            with nc.named_scope("proj"):
                for it in range(NT):
                    pt = ps_m.tile([P, KC, P], BF, tag="misc")
                    for k in range(KC):
                        nc.tensor.transpose(
                            pt[:, k, :], O_sb[it][:, k * P : (k + 1) * P], ident[:]
                        )
                    nc.scalar.copy(out=OT[:, :, it * P : (it + 1) * P], in_=pt[:])

                for it in range(NT):
                    ps = ps_acc.tile([P, C], F32, tag="acc")
                    for k in range(KC):
                        nc.tensor.matmul(
                            ps[:],
                            OT[:, k, it * P : (it + 1) * P],
                            wp_t[k][:],
                            start=(k == 0),
                            stop=(k == KC - 1) and not with_biases,
                        )
                    if with_biases:
                        nc.tensor.matmul(
                            ps[:], ones_row[:], pb_r[:], start=False, stop=True
                        )
                    xr = xr_pool.tile([P, C], F32, tag="xr")
                    nc.sync.dma_start(out=xr[:], in_=x_d[it * P : (it + 1) * P, :])
                    nc.vector.tensor_add(x1[it][:], xr[:], ps[:])

            # ---- phase 7: LN2 (reuses uT tiles) ----
            with nc.named_scope("ln2"):
                layernorm_to_uT(lambda it: x1[it], uT, ps_m2)

            # ---- phase 8+9: MLP, chunk-interleaved so MLP2 of chunk 0
            # overlaps MLP1 of chunk 1 ----
            def mlp1_chunk(nch):
                for fh in range(FH):
                    ps = ps_acc.tile([P, C], F32, tag="acc")
                    for k in range(KC):
                        nc.tensor.matmul(
                            ps[:],
                            w1_t[k][:, fh * P : (fh + 1) * P],
                            uT[:, k, nch * 512 : (nch + 1) * 512],
                            start=(k == 0), stop=(k == KC - 1),
                        )
                    nc.scalar.activation(
                        out=h1T[fh][:, nch * 512 : (nch + 1) * 512], in_=ps[:],
                        func=mybir.ActivationFunctionType.Gelu,
                        bias=m1b_t[:, fh : fh + 1], scale=1.0,
                    )

            def mlp2_tiles(its):
                for it in its:
                    ps = ps_acc.tile([P, C], F32, tag="acc")
                    for fh in range(FH):
                        nc.tensor.matmul(
                            ps[:],
                            h1T[fh][:, it * P : (it + 1) * P],
                            w2_t[fh][:],
                            start=(fh == 0),
                            stop=(fh == FH - 1) and not with_biases,
                        )
                    if with_biases:
                        nc.tensor.matmul(
                            ps[:], ones_row[:], m2b_r[:], start=False, stop=True
                        )
                    ot = out_pool.tile([P, C], F32, tag="ot")
                    nc.vector.tensor_add(ot[:], x1[it][:], ps[:])
                    nc.sync.dma_start(out=y_d[it * P : (it + 1) * P, :], in_=ot[:])

            with nc.named_scope("mlp"):
                mlp1_chunk(0)
                mlp2_tiles(range(0, 4))
                mlp1_chunk(1)
                mlp2_tiles(range(4, 8))

    _split_excess_waits(nc)
    return nc


_PROGRAMS = {}


def _get_program(with_biases: bool):
    if with_biases not in _PROGRAMS:
        _PROGRAMS[with_biases] = _build_program(with_biases)
    return _PROGRAMS[with_biases]


def _gelu_exact(x):
    from math import sqrt

    from numpy import vectorize  # noqa: F401  (erf via scipy-free path below)

    # exact erf-based gelu in float64
    import numpy as _np

    # erf via numpy's math: use vectorized math.erf for accuracy
    from math import erf

    ex = _np.vectorize(erf)(x / sqrt(2.0))
    return 0.5 * x * (1.0 + ex)


def _reference_np(x_token, wq, wk, wv, w_proj, b_proj, g1, b1, g2, b2,
                  w1, bb1, w2, bb2, idx):
    """float64 numpy fallback (used only if the banded-kernel preconditions
    fail, e.g. a cluster larger than MAX_CLUSTER tokens)."""
    x = x_token.astype(np.float64)
    out = np.empty_like(x)
    scale = HD ** -0.5
    for b in range(x.shape[0]):
        xb = x[b]
        mu = xb.mean(-1, keepdims=True)
        var = ((xb - mu) ** 2).mean(-1, keepdims=True)
        t = (xb - mu) / np.sqrt(var + LN_EPS) * g1 + b1
        q = (t @ wq.T).reshape(N, H, HD).transpose(1, 0, 2)
        k = (t @ wk.T).reshape(N, H, HD).transpose(1, 0, 2)
        v = (t @ wv.T).reshape(N, H, HD).transpose(1, 0, 2)
        s = np.einsum("hid,hjd->hij", q, k) * scale
        same = idx[b][None, :, None] == idx[b][None, None, :]
        e = np.exp(s) * same
        attn = (e + ATT_EPS / N) / (e.sum(-1, keepdims=True) + ATT_EPS)
        o = np.einsum("hij,hjd->hid", attn, v)
        o = o.transpose(1, 0, 2).reshape(N, C) @ w_proj.T + b_proj
        xr = xb + o
        mu = xr.mean(-1, keepdims=True)
        var = ((xr - mu) ** 2).mean(-1, keepdims=True)
        hh = (xr - mu) / np.sqrt(var + LN_EPS) * g2 + b2
        m = _gelu_exact(hh @ w1.T + bb1) @ w2.T + bb2
        out[b] = xr + m
    return out.astype(np.float32)


def kernel(**inputs):
    x_token = np.ascontiguousarray(np.asarray(inputs["x_token"], np.float32))
    idx = np.asarray(inputs["idx_cluster"]).astype(np.int64)
    wq = np.asarray(inputs["wq"], np.float32)
    wk = np.asarray(inputs["wk"], np.float32)
    wv = np.asarray(inputs["wv"], np.float32)
    w_proj = np.asarray(inputs["w_proj"], np.float32)
    b_proj = np.asarray(inputs["b_proj"], np.float32)
    g1 = np.asarray(inputs["g1"], np.float32)
    b1 = np.asarray(inputs["b1"], np.float32)
    g2 = np.asarray(inputs["g2"], np.float32)
    b2 = np.asarray(inputs["b2"], np.float32)
    w1 = np.asarray(inputs["w1"], np.float32)
    bb1 = np.asarray(inputs["bb1"], np.float32)
    w2 = np.asarray(inputs["w2"], np.float32)
    bb2 = np.asarray(inputs["bb2"], np.float32)

    perms = []
    ok = idx.min() >= 0 and idx.max() < CLN
    if ok:
        for b in range(B):
            sizes = np.bincount(idx[b], minlength=CLN)
            if sizes.max() > MAX_CLUSTER:
                ok = False
                break
    if not ok:
        return _reference_np(x_token, wq, wk, wv, w_proj, b_proj, g1, b1,
                             g2, b2, w1, bb1, w2, bb2, idx)

    # fold LN gains/biases into adjacent weights
    wqT = (g1[:, None] * wq.T).astype(BF16)
    wkT = (g1[:, None] * wk.T).astype(BF16)
    wvT = (g1[:, None] * wv.T).astype(BF16)
    wpT = np.ascontiguousarray(w_proj.T).astype(BF16)
    w1T = (g2[:, None] * w1.T).astype(BF16)
    w2T = np.ascontiguousarray(w2.T).astype(BF16)
    qb = (wq @ b1).astype(np.float32)
    kb = (wk @ b1).astype(np.float32)
    vb = (wv @ b1).astype(BF16)
    pb = b_proj.astype(BF16)
    m1b = (bb1 + w1 @ b2).astype(np.float32)
    m2b = bb2.astype(BF16)

    # g1/g2/bb1/b2 fold into weights and the gelu bias (m1b), so any values
    # are handled by the fast path. Nonzero b1/b_proj/bb2 would need the
    # untested bias program variant -- setup_inputs() hardcodes them to
    # zero, so route that (unreachable) case to the numpy fallback instead.
    if np.any(qb) or np.any(kb) or np.any(vb) or np.any(pb) or np.any(m2b):
        return _reference_np(x_token, wq, wk, wv, w_proj, b_proj, g1, b1,
                             g2, b2, w1, bb1, w2, bb2, idx)
    with_biases = False

    shared = dict(
        wqT=wqT, wkT=wkT, wvT=wvT, wpT=wpT, w1T=w1T, w2T=w2T, m1b=m1b,
    )

    in_maps = []
    ar = np.arange(CLN)
    for b in range(B):
        perm = np.argsort(idx[b], kind="stable")
        perms.append(perm)
        cid = idx[b][perm]
        onehot = (cid[None, :] == ar[:, None])
        in_maps.append(
            dict(
                shared,
                x=np.ascontiguousarray(x_token[b][perm]),
                qmask=(ALPHA_Q * onehot).astype(BF16),
                kmask=(ALPHA_K * onehot).astype(BF16),
            )
        )

    nc = _get_program(with_biases)
    res = run_bass_kernel_spmd(nc, in_maps, list(range(B)))
    global LAST_RESULTS
    LAST_RESULTS = res
    out = np.empty((B, N, C), np.float32)
    for b in range(B):
        out[b][perms[b]] = res.results[b]["y"]
    return out


LAST_RESULTS = None


# revision 52
# speedup vs baseline: 1.0008x; 1.0008x over previous
"""Trainium2 Bass kernel for nn_CBlock2 (sparse cluster attention block).

Strategy: data-parallel over batch B=8 across 8 cores. Per core, tokens are
host-sorted by cluster id so same-cluster attention pairs lie in a 3-tile
band around the diagonal; the cluster mask is folded into the score matmul
via one-hot augmentation rows (exp(s - 50) == 0 for cross-cluster pairs).
LayerNorm gains/biases are folded into the adjacent weight matrices on the
host. All matmuls run in bf16 with fp32 PSUM accumulation; the residual
stream stays fp32.
"""
import sys

sys.path.insert(0, "/opt/trn_rl_repo")

import numpy as np
import ml_dtypes

import concourse.bass as bass
import concourse.mybir as mybir
import concourse.tile as tile
from concourse.bass_utils import run_bass_kernel_spmd
from concourse.masks import make_identity

BF16 = ml_dtypes.bfloat16

B, N, C, H, PD, CLN = 8, 1024, 512, 8, 256, 64
HD = C // H          # 64
HID = 4 * C          # 2048
LN_EPS = 1e-5
ATT_EPS = 1e-6
P = 128
NT = N // P          # 8 token tiles
KC = C // P          # 4 contraction tiles over C
FH = HID // P        # 16 hidden tiles
SCALE = HD ** -0.5   # 0.125
ALPHA_Q = 16.0
ALPHA_K = 25.0
BIG = ALPHA_Q * ALPHA_K * SCALE  # 50.0: mask margin inside exp
MAX_CLUSTER = 65     # band bound: all same-cluster pairs within +-64 tokens

F32 = mybir.dt.float32
BF = mybir.dt.bfloat16


def _split_excess_waits(nc, max_waits=1):
    """walrus in this env rejects >1 sync-wait on one instruction; hoist
    excess waits onto same-engine no-op carriers inserted just before."""
    for f in nc.m.functions:
        for bb in f.blocks:
            new_insts = []
            for inst in bb.instructions:
                si = inst.sync_info
                if si is not None and si.on_wait and len(si.on_wait) > max_waits:
                    waits = list(si.on_wait)
                    excess, keep = waits[:-max_waits], waits[-max_waits:]
                    for ci in range(0, len(excess), max_waits):
                        chunk = excess[ci : ci + max_waits]
                        new_insts.append(
                            mybir.InstNoOp(
                                name=f"{inst.name}-ws{ci}",
                                engine=inst.engine,
                                ins=[],
                                outs=[],
                                sync_info=mybir.SyncInfo(on_wait=chunk, on_update=[]),
                            )
                        )
                    inst.sync_info = mybir.SyncInfo(
                        on_wait=keep, on_update=list(si.on_update)
                    )
                new_insts.append(inst)
            bb.instructions = new_insts


_PROGRAM = None


def _band_range(jt):
    """i-range covered by the score tile of j-tile jt (+-64-token band)."""
    i0 = max(0, jt * P - 64)
    i1 = min(N, (jt + 1) * P + 64)
    return i0, i1


def _build_program(with_biases: bool):
    """Build the per-core SPMD program. with_biases=False omits the bias
    paths (all reference biases are zero in the standard setup)."""
    nc = bass.Bass()

    x_d = nc.declare_dram_parameter("x", [N, C], F32, isOutput=False)
    wqT_d = nc.declare_dram_parameter("wqT", [C, C], BF, isOutput=False)
    wkT_d = nc.declare_dram_parameter("wkT", [C, C], BF, isOutput=False)
    wvT_d = nc.declare_dram_parameter("wvT", [C, C], BF, isOutput=False)
    wpT_d = nc.declare_dram_parameter("wpT", [C, C], BF, isOutput=False)
    w1T_d = nc.declare_dram_parameter("w1T", [C, HID], BF, isOutput=False)
    w2T_d = nc.declare_dram_parameter("w2T", [HID, C], BF, isOutput=False)
    m1b_d = nc.declare_dram_parameter("m1b", [HID], F32, isOutput=False)
    qm_d = nc.declare_dram_parameter("qmask", [CLN, N], BF, isOutput=False)
    km_d = nc.declare_dram_parameter("kmask", [CLN, N], BF, isOutput=False)
    if with_biases:
        qb_d = nc.declare_dram_parameter("qb", [C], F32, isOutput=False)
        kb_d = nc.declare_dram_parameter("kb", [C], F32, isOutput=False)
        vb_d = nc.declare_dram_parameter("vb", [C], BF, isOutput=False)
        pb_d = nc.declare_dram_parameter("pb", [C], BF, isOutput=False)
        m2b_d = nc.declare_dram_parameter("m2b", [C], BF, isOutput=False)
    y_d = nc.declare_dram_parameter("y", [N, C], F32, isOutput=True)

    with tile.TileContext(nc) as tc:
        from contextlib import ExitStack

        with ExitStack() as ctx:
            ec = ctx.enter_context
            persist = ec(tc.tile_pool(name="persist", bufs=1))
            w_small = ec(tc.tile_pool(name="w_small", bufs=1))
            w_big = ec(tc.tile_pool(name="w_big", bufs=1))
            xs_pool = ec(tc.tile_pool(name="xs", bufs=4))
            xr_pool = ec(tc.tile_pool(name="xr", bufs=2))
            ln_pool = ec(tc.tile_pool(name="ln", bufs=6))
            e_pool = ec(tc.tile_pool(name="epool", bufs=6))
            r_pool = ec(tc.tile_pool(name="rpool", bufs=8))
            out_pool = ec(tc.tile_pool(name="outp", bufs=3))

            # ---- tiny constants (cheap, emitted first) ----
            ident = persist.tile([P, P], BF, tag="ident")
            make_identity(nc, ident[:])
            eps_t = persist.tile([P, 1], F32, tag="eps_t")
            nc.vector.memset(eps_t[:], LN_EPS)
            nbig_t = persist.tile([P, 1], F32, tag="nbig_t")
            nc.vector.memset(nbig_t[:], -BIG)
            if with_biases:
                ones_row = persist.tile([1, P], BF, tag="ones_row")
                nc.vector.memset(ones_row[:], 1.0)

            # ---- persistent activations ----
            # qA/kA: one wide tile, head h at free cols [h*N,(h+1)*N). Even
            # heads keep q rows on partitions 0:64 + mask rows on 64:128;
            # odd heads are flipped so PSUM->SBUF copies stay
            # partition-aligned.
            qA = persist.tile([P, H * N], BF, tag="qA")
            kA = persist.tile([P, H * N], BF, tag="kA")
            vext = [persist.tile([P, H, HD + 1], BF, tag=f"vx{t}", name=f"vx{t}") for t in range(NT)]
            uT = persist.tile([P, KC, N], BF, tag="uT")
            OT = persist.tile([P, KC, N], BF, tag="OT")
            O_sb = [persist.tile([P, C], BF, tag=f"Osb{t}", name=f"Osb{t}") for t in range(NT)]
            x1 = [persist.tile([P, C], F32, tag=f"x1{t}", name=f"x1{t}") for t in range(NT)]
            h1T = [persist.tile([P, N], BF, tag=f"h1T{k}", name=f"h1T{k}") for k in range(FH)]

            wq_t = [w_small.tile([P, C], BF, tag=f"wq{k}", name=f"wq{k}") for k in range(KC)]
            wk_t = [w_small.tile([P, C], BF, tag=f"wk{k}", name=f"wk{k}") for k in range(KC)]
            wv_t = [w_small.tile([P, C], BF, tag=f"wv{k}", name=f"wv{k}") for k in range(KC)]
            wp_t = [w_small.tile([P, C], BF, tag=f"wp{k}", name=f"wp{k}") for k in range(KC)]
            w2_t = [w_small.tile([P, C], BF, tag=f"w2{k}", name=f"w2{k}") for k in range(FH)]
            w1_t = [w_big.tile([P, HID], BF, tag=f"w1{k}", name=f"w1{k}") for k in range(KC)]

            def layernorm_to_uT(src_tile_fn, dst_uT, ps_m):
                """token-major f32 tiles -> normalized bf16, PE-transposed into
                feature-major dst_uT (KC tiles of [P, N])."""
                for it in range(NT):
                    xt = src_tile_fn(it)
                    stats = ln_pool.tile([P, 6], F32, tag="stats")
                    nc.vector.bn_stats(out=stats[:], in_=xt[:])
                    mv = ln_pool.tile([P, 2], F32, tag="mv")
                    nc.vector.bn_aggr(out=mv[:], in_=stats[:])
                    std = ln_pool.tile([P, 1], F32, tag="std")
                    nc.scalar.activation(
                        out=std[:], in_=mv[:, 1:2],
                        func=mybir.ActivationFunctionType.Sqrt,
                        bias=eps_t[:], scale=1.0,
                    )
                    nc.vector.reciprocal(out=std[:], in_=std[:])
                    u = ln_pool.tile([P, C], BF, tag="u")
                    nc.vector.tensor_scalar(
                        out=u[:], in0=xt[:],
                        scalar1=mv[:, 0:1], scalar2=std[:],
                        op0=mybir.AluOpType.subtract, op1=mybir.AluOpType.mult,
                    )
                    pt = ps_m.tile([P, KC, P], BF, tag="misc")
                    for k in range(KC):
                        nc.tensor.transpose(
                            pt[:, k, :], u[:, k * P : (k + 1) * P], ident[:]
                        )
                    nc.scalar.copy(
                        out=dst_uT[:, :, it * P : (it + 1) * P], in_=pt[:]
                    )

            # ---- phase 1: LN1 (x streamed in first -- nothing queues ahead) ----
            def _x_src(it):
                xt = xs_pool.tile([P, C], F32, tag="xt")
                nc.sync.dma_start(out=xt[:], in_=x_d[it * P : (it + 1) * P, :])
                return xt

            with nc.named_scope("ln1"), \
                    tc.tile_pool(name="ps_m1", bufs=2, space="PSUM") as ps_m1:
                layernorm_to_uT(_x_src, uT, ps_m1)

            # qkv weights arrive while LN1 runs
            for k in range(KC):
                nc.sync.dma_start(out=wq_t[k][:], in_=wqT_d[k * P : (k + 1) * P, :])
            for k in range(KC):
                nc.sync.dma_start(out=wk_t[k][:], in_=wkT_d[k * P : (k + 1) * P, :])
            for k in range(KC):
                nc.sync.dma_start(out=wv_t[k][:], in_=wvT_d[k * P : (k + 1) * P, :])
            for mk, dstA in ((qm_d, qA), (km_d, kA)):
                map_ = mk[:]
                src_b = bass.AP(
                    tensor=map_.tensor, offset=map_.offset,
                    ap=[map_.ap[0], [0, H // 2], map_.ap[1]],
                )
                ev = dstA[:].rearrange("p (h n) -> p h n", h=H)
                nc.sync.dma_start(out=ev[HD:P, 0:H:2, :], in_=src_b)
                nc.sync.dma_start(out=ev[0:HD, 1:H:2, :], in_=src_b)
            m1b_t = persist.tile([P, FH], F32, tag="m1b")
            nc.sync.dma_start(out=m1b_t[:], in_=m1b_d.rearrange("(f p) -> p f", p=P))
            if with_biases:
                qb_t = persist.tile([P, KC], F32, tag="qb")
                nc.sync.dma_start(
                    out=qb_t[:], in_=qb_d.rearrange("(c p) -> p c", p=P)
                )
                kb_t = persist.tile([P, KC], F32, tag="kb")
                nc.sync.dma_start(
                    out=kb_t[:], in_=kb_d.rearrange("(c p) -> p c", p=P)
                )
                vb_r = persist.tile([1, C], BF, tag="vb_r")
                nc.sync.dma_start(out=vb_r[:], in_=vb_d.rearrange("(a c) -> a c", a=1))
                pb_r = persist.tile([1, C], BF, tag="pb_r")
                nc.sync.dma_start(out=pb_r[:], in_=pb_d.rearrange("(a c) -> a c", a=1))
                m2b_r = persist.tile([1, C], BF, tag="m2b_r")
                nc.sync.dma_start(
                    out=m2b_r[:], in_=m2b_d.rearrange("(a c) -> a c", a=1)
                )

            # ---- phase 2: QKV ----
            # q/k feature-major into augmented head tiles (partitions 0:64):
            # q copies on ACT, k copies on DVE to balance engine load.
            with nc.named_scope("qkv"), \
                    tc.tile_pool(name="ps_qkv", bufs=6, space="PSUM") as ps_acc:
                for which, w_t, dstA in (("q", wq_t, qA), ("k", wk_t, kA)):
                    for co in range(KC):  # output-feature tile: heads 2co, 2co+1
                        for nch in range(2):  # 512-token chunks
                            ps = ps_acc.tile([P, C], F32, tag="acc")
                            for k in range(KC):
                                nc.tensor.matmul(
                                    ps[:],
                                    w_t[k][:, co * P : (co + 1) * P],
                                    uT[:, k, nch * 512 : (nch + 1) * 512],
                                    start=(k == 0), stop=(k == KC - 1),
                                )
                            for half in range(2):
                                h = 2 * co + half
                                pr = slice(half * HD, (half + 1) * HD)
                                dst = dstA[pr, h * N + nch * 512 : h * N + (nch + 1) * 512]
                                src = ps[pr, :]
                                if which == "q":
                                    if with_biases:
                                        nc.scalar.activation(
                                            out=dst, in_=src,
                                            func=mybir.ActivationFunctionType.Identity,
                                            bias=qb_t[half * HD : (half + 1) * HD, co : co + 1],
                                            scale=1.0,
                                        )
                                    else:
                                        nc.scalar.copy(out=dst, in_=src)
                                else:
                                    if with_biases:
                                        nc.vector.tensor_scalar_add(
                                            dst, src,
                                            kb_t[half * HD : (half + 1) * HD, co : co + 1],
                                        )
                                    elif (co + nch) % 2 == 0:
                                        nc.vector.tensor_copy(out=dst, in_=src)
                                    else:
                                        nc.scalar.copy(out=dst, in_=src)
                # v token-major packed into vext (+ ones column for denominators)
                for it in range(NT):
                    ps = ps_acc.tile([P, C], F32, tag="acc")
                    for k in range(KC):
                        nc.tensor.matmul(
                            ps[:],
                            uT[:, k, it * P : (it + 1) * P],
                            wv_t[k][:],
                            start=(k == 0),
                            stop=(k == KC - 1) and not with_biases,
                        )
                    if with_biases:
                        nc.tensor.matmul(
                            ps[:], ones_row[:], vb_r[:], start=False, stop=True
                        )
                    vx = vext[it]
                    nc.gpsimd.memset(vx[:, :, HD : HD + 1], 1.0)
                    nc.vector.tensor_copy(
                        out=vx[:, :, 0:HD],
                        in_=ps[:].rearrange("p (h d) -> p h d", h=H),
                    )

            # proj weights arrive during QKV/attention
            for k in range(KC):
                nc.sync.dma_start(out=wp_t[k][:], in_=wpT_d[k * P : (k + 1) * P, :])

            # ---- phase 3+4: per-head banded scores/exp then AV ----
            # score tiles for j-tile pairs (2jp, 2jp+1) share one PSUM tile:
            # pair widths are <=512 so both sub-tiles fit one PSUM bank and
            # exp runs as one wide activation per pair.
            pair_off = {}
            for jt in range(NT):
                i0, i1 = _band_range(jt)
                pair_off[jt] = 0 if jt % 2 == 0 else (
                    _band_range(jt - 1)[1] - _band_range(jt - 1)[0]
                )
            with nc.named_scope("attn"), \
                    tc.tile_pool(name="ps_sc", bufs=2, space="PSUM") as ps_s, \
                    tc.tile_pool(name="ps_po", bufs=4, space="PSUM") as ps_m:
                # quad offsets: jts 4g..4g+3 share one 2-bank psum tile; subs
                # 0,1 pack into bank0 and 2,3 into bank1 (a matmul output
                # must stay within one PSUM bank).
                qoff = {}
                for jt in range(NT):
                    g, sub = jt // 4, jt % 4
                    if sub == 0:
                        qoff[jt] = 0
                    elif sub == 1:
                        qoff[jt] = _band_range(jt - 1)[1] - _band_range(jt - 1)[0]
                    elif sub == 2:
                        qoff[jt] = 512
                    else:
                        qoff[jt] = 512 + _band_range(jt - 1)[1] - _band_range(jt - 1)[0]
                for h in range(H):
                    e_tiles = {}
                    for g in range(NT // 4):
                        jlast = 4 * g + 3
                        wtot = qoff[jlast] + _band_range(jlast)[1] - _band_range(jlast)[0]
                        ps = ps_s.tile([P, 1024], F32, tag="sc")
                        for sub in range(4):
                            jt = 4 * g + sub
                            i0, i1 = _band_range(jt)
                            nc.tensor.matmul(
                                ps[:, qoff[jt] : qoff[jt] + (i1 - i0)],
                                kA[:, h * N + jt * P : h * N + (jt + 1) * P],
                                qA[:, h * N + i0 : h * N + i1],
                                start=True, stop=True,
                            )
                        et = e_pool.tile([P, 1024], BF, tag="et")
                        nc.scalar.activation(
                            out=et[:, 0:wtot], in_=ps[:, 0:wtot],
                            func=mybir.ActivationFunctionType.Exp,
                            bias=nbig_t[:], scale=SCALE,
                        )
                        e_tiles[g] = et
                    for it in range(NT):
                        po = ps_m.tile([P, HD + 1], F32, tag="misc")
                        # center tile zero-initializes the whole psum region;
                        # the +-64 edge pieces accumulate partial partitions.
                        c_off = qoff[it] + it * P - _band_range(it)[0]
                        nc.tensor.matmul(
                            po[:],
                            e_tiles[it // 4][:, c_off : c_off + P],
                            vext[it][:, h, :],
                            start=True, stop=False,
                        )
                        pieces = []
                        if it > 0:
                            jt = it - 1
                            off = qoff[jt] + it * P - _band_range(jt)[0]
                            pieces.append((jt, off, 0))      # po[0:64]
                        if it < NT - 1:
                            jt = it + 1
                            off = qoff[jt]                   # i starts at its i0
                            pieces.append((jt, off, HD))     # po[64:128]
                        for n_, (jt, off, pbase) in enumerate(pieces):
                            nc.tensor.matmul(
                                po[pbase : pbase + HD, :],
                                e_tiles[jt // 4][:, off : off + HD],
                                vext[jt][:, h, :],
                                start=False, stop=(n_ == len(pieces) - 1),
                            )
                        # D >= ~0.4 always, so the reference's +1e-6 in the
                        # denominator is far below bf16 noise; skip it.
                        r = r_pool.tile([P, 1], F32, tag="r")
                        nc.vector.reciprocal(r[:], po[:, HD : HD + 1])
                        nc.vector.tensor_scalar_mul(
                            O_sb[it][:, h * HD : (h + 1) * HD], po[:, 0:HD], r[:]
                        )

            # mlp weights arrive during attention
            for k in range(KC):
                nc.sync.dma_start(out=w1_t[k][:], in_=w1T_d[k * P : (k + 1) * P, :])
            for k in range(FH):
                nc.sync.dma_start(out=w2_t[k][:], in_=w2T_d[k * P : (k + 1) * P, :])

            # ---- phase 5: transpose O + proj + residual ----
            ps_m2 = ec(tc.tile_pool(name="ps_m2", bufs=2, space="PSUM"))
            ps_acc = ec(tc.tile_pool(name="ps_acc2", bufs=5, space="PSUM"))
            ps_m = ps_m2
            with nc.named_scope("proj"):
                for it in range(NT):
                    pt = ps_m.tile([P, KC, P], BF, tag="misc")
                    for k in range(KC):
                        nc.tensor.transpose(
                            pt[:, k, :], O_sb[it][:, k * P : (k + 1) * P], ident[:]
                        )
                    nc.scalar.copy(out=OT[:, :, it * P : (it + 1) * P], in_=pt[:])

                for it in range(NT):
                    ps = ps_acc.tile([P, C], F32, tag="acc")
                    for k in range(KC):
                        nc.tensor.matmul(
                            ps[:],
                            OT[:, k, it * P : (it + 1) * P],
                            wp_t[k][:],
                            start=(k == 0),
                            stop=(k == KC - 1) and not with_biases,
                        )
                    if with_biases:
                        nc.tensor.matmul(
                            ps[:], ones_row[:], pb_r[:], start=False, stop=True
                        )
                    xr = xr_pool.tile([P, C], F32, tag="xr")
                    nc.sync.dma_start(out=xr[:], in_=x_d[it * P : (it + 1) * P, :])
                    nc.vector.tensor_add(x1[it][:], xr[:], ps[:])

            # ---- phase 7: LN2 (reuses uT tiles) ----
            with nc.named_scope("ln2"):
                layernorm_to_uT(lambda it: x1[it], uT, ps_m2)

            # ---- phase 8+9: MLP, chunk-interleaved so MLP2 of chunk 0
            # overlaps MLP1 of chunk 1 ----
            def mlp1_chunk(nch):
                for fh in range(FH):
                    ps = ps_acc.tile([P, C], F32, tag="acc")
                    for k in range(KC):
                        nc.tensor.matmul(
                            ps[:],
                            w1_t[k][:, fh * P : (fh + 1) * P],
                            uT[:, k, nch * 512 : (nch + 1) * 512],
                            start=(k == 0), stop=(k == KC - 1),
                        )
                    nc.scalar.activation(
                        out=h1T[fh][:, nch * 512 : (nch + 1) * 512], in_=ps[:],
                        func=mybir.ActivationFunctionType.Gelu,
                        bias=m1b_t[:, fh : fh + 1], scale=1.0,
                    )

            def mlp2_tiles(its):
                for it in its:
                    ps = ps_acc.tile([P, C], F32, tag="acc")
                    for fh in range(FH):
                        nc.tensor.matmul(
                            ps[:],
                            h1T[fh][:, it * P : (it + 1) * P],
                            w2_t[fh][:],
                            start=(fh == 0),
                            stop=(fh == FH - 1) and not with_biases,
                        )
                    if with_biases:
                        nc.tensor.matmul(
                            ps[:], ones_row[:], m2b_r[:], start=False, stop=True
                        )
                    ot = out_pool.tile([P, C], F32, tag="ot")
                    nc.vector.tensor_add(ot[:], x1[it][:], ps[:])
                    nc.sync.dma_start(out=y_d[it * P : (it + 1) * P, :], in_=ot[:])

            with nc.named_scope("mlp"):
                mlp1_chunk(0)
                mlp2_tiles(range(0, 4))
                mlp1_chunk(1)
                mlp2_tiles(range(4, 8))

    _split_excess_waits(nc)
    return nc


_PROGRAMS = {}


def _get_program(with_biases: bool):
    if with_biases not in _PROGRAMS:
        _PROGRAMS[with_biases] = _build_program(with_biases)
    return _PROGRAMS[with_biases]


def _gelu_exact(x):
    from math import sqrt

    from numpy import vectorize  # noqa: F401  (erf via scipy-free path below)

    # exact erf-based gelu in float64
    import numpy as _np

    # erf via numpy's math: use vectorized math.erf for accuracy
    from math import erf

    ex = _np.vectorize(erf)(x / sqrt(2.0))
    return 0.5 * x * (1.0 + ex)


def _reference_np(x_token, wq, wk, wv, w_proj, b_proj, g1, b1, g2, b2,
                  w1, bb1, w2, bb2, idx):
    """float64 numpy fallback (used only if the banded-kernel preconditions
    fail, e.g. a cluster larger than MAX_CLUSTER tokens)."""
    x = x_token.astype(np.float64)
    out = np.empty_like(x)
    scale = HD ** -0.5
    for b in range(x.shape[0]):
        xb = x[b]
        mu = xb.mean(-1, keepdims=True)
        var = ((xb - mu) ** 2).mean(-1, keepdims=True)
        t = (xb - mu) / np.sqrt(var + LN_EPS) * g1 + b1
        q = (t @ wq.T).reshape(N, H, HD).transpose(1, 0, 2)
        k = (t @ wk.T).reshape(N, H, HD).transpose(1, 0, 2)
        v = (t @ wv.T).reshape(N, H, HD).transpose(1, 0, 2)
        s = np.einsum("hid,hjd->hij", q, k) * scale
        same = idx[b][None, :, None] == idx[b][None, None, :]
        e = np.exp(s) * same
        attn = (e + ATT_EPS / N) / (e.sum(-1, keepdims=True) + ATT_EPS)
        o = np.einsum("hij,hjd->hid", attn, v)
        o = o.transpose(1, 0, 2).reshape(N, C) @ w_proj.T + b_proj
        xr = xb + o
        mu = xr.mean(-1, keepdims=True)
        var = ((xr - mu) ** 2).mean(-1, keepdims=True)
        hh = (xr - mu) / np.sqrt(var + LN_EPS) * g2 + b2
        m = _gelu_exact(hh @ w1.T + bb1) @ w2.T + bb2
        out[b] = xr + m
    return out.astype(np.float32)


def kernel(**inputs):
    x_token = np.ascontiguousarray(np.asarray(inputs["x_token"], np.float32))
    idx = np.asarray(inputs["idx_cluster"]).astype(np.int64)
    wq = np.asarray(inputs["wq"], np.float32)
    wk = np.asarray(inputs["wk"], np.float32)
    wv = np.asarray(inputs["wv"], np.float32)
    w_proj = np.asarray(inputs["w_proj"], np.float32)
    b_proj = np.asarray(inputs["b_proj"], np.float32)
    g1 = np.asarray(inputs["g1"], np.float32)
    b1 = np.asarray(inputs["b1"], np.float32)
    g2 = np.asarray(inputs["g2"], np.float32)
    b2 = np.asarray(inputs["b2"], np.float32)
    w1 = np.asarray(inputs["w1"], np.float32)
    bb1 = np.asarray(inputs["bb1"], np.float32)
    w2 = np.asarray(inputs["w2"], np.float32)
    bb2 = np.asarray(inputs["bb2"], np.float32)

    perms = []
    ok = idx.min() >= 0 and idx.max() < CLN
    if ok:
        for b in range(B):
            sizes = np.bincount(idx[b], minlength=CLN)
            if sizes.max() > MAX_CLUSTER:
                ok = False
                break
    if not ok:
        return _reference_np(x_token, wq, wk, wv, w_proj, b_proj, g1, b1,
                             g2, b2, w1, bb1, w2, bb2, idx)

    # fold LN gains/biases into adjacent weights
    wqT = (g1[:, None] * wq.T).astype(BF16)
    wkT = (g1[:, None] * wk.T).astype(BF16)
    wvT = (g1[:, None] * wv.T).astype(BF16)
    wpT = np.ascontiguousarray(w_proj.T).astype(BF16)
    w1T = (g2[:, None] * w1.T).astype(BF16)
    w2T = np.ascontiguousarray(w2.T).astype(BF16)
    qb = (wq @ b1).astype(np.float32)
    kb = (wk @ b1).astype(np.float32)
    vb = (wv @ b1).astype(BF16)
    pb = b_proj.astype(BF16)
    m1b = (bb1 + w1 @ b2).astype(np.float32)
    m2b = bb2.astype(BF16)

    # g1/g2/bb1/b2 fold into weights and the gelu bias (m1b), so any values
    # are handled by the fast path. Nonzero b1/b_proj/bb2 would need the
    # untested bias program variant -- setup_inputs() hardcodes them to
    # zero, so route that (unreachable) case to the numpy fallback instead.
    if np.any(qb) or np.any(kb) or np.any(vb) or np.any(pb) or np.any(m2b):
        return _reference_np(x_token, wq, wk, wv, w_proj, b_proj, g1, b1,
                             g2, b2, w1, bb1, w2, bb2, idx)
    with_biases = False

    shared = dict(
        wqT=wqT, wkT=wkT, wvT=wvT, wpT=wpT, w1T=w1T, w2T=w2T, m1b=m1b,
    )

    in_maps = []
    ar = np.arange(CLN)
    for b in range(B):
        perm = np.argsort(idx[b], kind="stable")
        perms.append(perm)
        cid = idx[b][perm]
        onehot = (cid[None, :] == ar[:, None])
        in_maps.append(
            dict(
                shared,
                x=np.ascontiguousarray(x_token[b][perm]),
                qmask=(ALPHA_Q * onehot).astype(BF16),
                kmask=(ALPHA_K * onehot).astype(BF16),
            )
        )

    nc = _get_program(with_biases)
    res = run_bass_kernel_spmd(nc, in_maps, list(range(B)))
    global LAST_RESULTS
    LAST_RESULTS = res
    out = np.empty((B, N, C), np.float32)
    for b in range(B):
        out[b][perms[b]] = res.results[b]["y"]
    return out


LAST_RESULTS = None


# revision 57
# speedup vs baseline: 1.0088x; 1.0080x over previous
"""Trainium2 Bass kernel for nn_CBlock2 (sparse cluster attention block).

Strategy: data-parallel over batch B=8 across 8 cores. Per core, tokens are
host-sorted by cluster id so same-cluster attention pairs lie in a 3-tile
band around the diagonal; the cluster mask is folded into the score matmul
via one-hot augmentation rows (exp(s - 50) == 0 for cross-cluster pairs).
LayerNorm gains/biases are folded into the adjacent weight matrices on the
host. All matmuls run in bf16 with fp32 PSUM accumulation; the residual
stream stays fp32.
"""
import sys

sys.path.insert(0, "/opt/trn_rl_repo")

import numpy as np
import ml_dtypes

import concourse.bass as bass
import concourse.mybir as mybir
import concourse.tile as tile
from concourse.bass_utils import run_bass_kernel_spmd
from concourse.masks import make_identity

BF16 = ml_dtypes.bfloat16

B, N, C, H, PD, CLN = 8, 1024, 512, 8, 256, 64
HD = C // H          # 64
HID = 4 * C          # 2048
LN_EPS = 1e-5
ATT_EPS = 1e-6
P = 128
NT = N // P          # 8 token tiles
KC = C // P          # 4 contraction tiles over C
FH = HID // P        # 16 hidden tiles
SCALE = HD ** -0.5   # 0.125
ALPHA_Q = 16.0
ALPHA_K = 25.0
BIG = ALPHA_Q * ALPHA_K * SCALE  # 50.0: mask margin inside exp
MAX_CLUSTER = 65     # band bound: all same-cluster pairs within +-64 tokens

F32 = mybir.dt.float32
BF = mybir.dt.bfloat16


def _split_excess_waits(nc, max_waits=1):
    """walrus in this env rejects >1 sync-wait on one instruction; hoist
    excess waits onto same-engine no-op carriers inserted just before."""
    for f in nc.m.functions:
        for bb in f.blocks:
            new_insts = []
            for inst in bb.instructions:
                si = inst.sync_info
                if si is not None and si.on_wait and len(si.on_wait) > max_waits:
                    waits = list(si.on_wait)
                    excess, keep = waits[:-max_waits], waits[-max_waits:]
                    for ci in range(0, len(excess), max_waits):
                        chunk = excess[ci : ci + max_waits]
                        new_insts.append(
                            mybir.InstNoOp(
                                name=f"{inst.name}-ws{ci}",
                                engine=inst.engine,
                                ins=[],
                                outs=[],
                                sync_info=mybir.SyncInfo(on_wait=chunk, on_update=[]),
                            )
                        )
                    inst.sync_info = mybir.SyncInfo(
                        on_wait=keep, on_update=list(si.on_update)
                    )
                new_insts.append(inst)
            bb.instructions = new_insts


_PROGRAM = None


def _band_range(jt):
    """i-range covered by the score tile of j-tile jt (+-64-token band)."""
    i0 = max(0, jt * P - 64)
    i1 = min(N, (jt + 1) * P + 64)
    return i0, i1


def _build_program(with_biases: bool):
    """Build the per-core SPMD program. with_biases=False omits the bias
    paths (all reference biases are zero in the standard setup)."""
    nc = bass.Bass()

    x_d = nc.declare_dram_parameter("x", [N, C], F32, isOutput=False)
    wqT_d = nc.declare_dram_parameter("wqT", [C, C], BF, isOutput=False)
    wkT_d = nc.declare_dram_parameter("wkT", [C, C], BF, isOutput=False)
    wvT_d = nc.declare_dram_parameter("wvT", [C, C], BF, isOutput=False)
    wpT_d = nc.declare_dram_parameter("wpT", [C, C], BF, isOutput=False)
    w1T_d = nc.declare_dram_parameter("w1T", [C, HID], BF, isOutput=False)
    w2T_d = nc.declare_dram_parameter("w2T", [HID, C], BF, isOutput=False)
    m1b_d = nc.declare_dram_parameter("m1b", [HID], F32, isOutput=False)
    qm_d = nc.declare_dram_parameter("qmask", [CLN, N], BF, isOutput=False)
    km_d = nc.declare_dram_parameter("kmask", [CLN, N], BF, isOutput=False)
    if with_biases:
        qb_d = nc.declare_dram_parameter("qb", [C], F32, isOutput=False)
        kb_d = nc.declare_dram_parameter("kb", [C], F32, isOutput=False)
        vb_d = nc.declare_dram_parameter("vb", [C], BF, isOutput=False)
        pb_d = nc.declare_dram_parameter("pb", [C], BF, isOutput=False)
        m2b_d = nc.declare_dram_parameter("m2b", [C], BF, isOutput=False)
    y_d = nc.declare_dram_parameter("y", [N, C], F32, isOutput=True)

    with tile.TileContext(nc) as tc:
        from contextlib import ExitStack

        with ExitStack() as ctx:
            ec = ctx.enter_context
            persist = ec(tc.tile_pool(name="persist", bufs=1))
            w_small = ec(tc.tile_pool(name="w_small", bufs=1))
            w_big = ec(tc.tile_pool(name="w_big", bufs=1))
            xs_pool = ec(tc.tile_pool(name="xs", bufs=4))
            xr_pool = ec(tc.tile_pool(name="xr", bufs=3))
            ln_pool = ec(tc.tile_pool(name="ln", bufs=6))
            e_pool = ec(tc.tile_pool(name="epool", bufs=6))
            r_pool = ec(tc.tile_pool(name="rpool", bufs=8))
            out_pool = ec(tc.tile_pool(name="outp", bufs=3))

            # ---- tiny constants (cheap, emitted first) ----
            ident = persist.tile([P, P], BF, tag="ident")
            make_identity(nc, ident[:])
            eps_t = persist.tile([P, 1], F32, tag="eps_t")
            nc.vector.memset(eps_t[:], LN_EPS)
            nbig_t = persist.tile([P, 1], F32, tag="nbig_t")
            nc.vector.memset(nbig_t[:], -BIG)
            if with_biases:
                ones_row = persist.tile([1, P], BF, tag="ones_row")
                nc.vector.memset(ones_row[:], 1.0)

            # ---- persistent activations ----
            # qA/kA: one wide tile, head h at free cols [h*N,(h+1)*N). Even
            # heads keep q rows on partitions 0:64 + mask rows on 64:128;
            # odd heads are flipped so PSUM->SBUF copies stay
            # partition-aligned.
            qA = persist.tile([P, H * N], BF, tag="qA")
            kA = persist.tile([P, H * N], BF, tag="kA")
            vext = [persist.tile([P, H, HD + 1], BF, tag=f"vx{t}", name=f"vx{t}") for t in range(NT)]
            uT = persist.tile([P, KC, N], BF, tag="uT")
            OT = persist.tile([P, KC, N], BF, tag="OT")
            O_sb = [persist.tile([P, C], BF, tag=f"Osb{t}", name=f"Osb{t}") for t in range(NT)]
            x1 = [persist.tile([P, C], F32, tag=f"x1{t}", name=f"x1{t}") for t in range(NT)]
            h1T = [persist.tile([P, N], BF, tag=f"h1T{k}", name=f"h1T{k}") for k in range(FH)]

            wq_t = [w_small.tile([P, C], BF, tag=f"wq{k}", name=f"wq{k}") for k in range(KC)]
            wk_t = [w_small.tile([P, C], BF, tag=f"wk{k}", name=f"wk{k}") for k in range(KC)]
            wv_t = [w_small.tile([P, C], BF, tag=f"wv{k}", name=f"wv{k}") for k in range(KC)]
            wp_t = [w_small.tile([P, C], BF, tag=f"wp{k}", name=f"wp{k}") for k in range(KC)]
            w2_t = [w_small.tile([P, C], BF, tag=f"w2{k}", name=f"w2{k}") for k in range(FH)]
            w1_t = [w_big.tile([P, HID], BF, tag=f"w1{k}", name=f"w1{k}") for k in range(KC)]

            def layernorm_to_uT(src_tile_fn, dst_uT, ps_m):
                """token-major f32 tiles -> normalized bf16, PE-transposed into
                feature-major dst_uT (KC tiles of [P, N])."""
                for it in range(NT):
                    xt = src_tile_fn(it)
                    stats = ln_pool.tile([P, 6], F32, tag="stats")
                    nc.vector.bn_stats(out=stats[:], in_=xt[:])
                    mv = ln_pool.tile([P, 2], F32, tag="mv")
                    nc.vector.bn_aggr(out=mv[:], in_=stats[:])
                    std = ln_pool.tile([P, 1], F32, tag="std")
                    nc.scalar.activation(
                        out=std[:], in_=mv[:, 1:2],
                        func=mybir.ActivationFunctionType.Sqrt,
                        bias=eps_t[:], scale=1.0,
                    )
                    nc.vector.reciprocal(out=std[:], in_=std[:])
                    u = ln_pool.tile([P, C], BF, tag="u")
                    nc.vector.tensor_scalar(
                        out=u[:], in0=xt[:],
                        scalar1=mv[:, 0:1], scalar2=std[:],
                        op0=mybir.AluOpType.subtract, op1=mybir.AluOpType.mult,
                    )
                    pt = ps_m.tile([P, KC, P], BF, tag="misc")
                    for k in range(KC):
                        nc.tensor.transpose(
                            pt[:, k, :], u[:, k * P : (k + 1) * P], ident[:]
                        )
                    nc.scalar.copy(
                        out=dst_uT[:, :, it * P : (it + 1) * P], in_=pt[:]
                    )

            # ---- phase 1: LN1 (x streamed in first -- nothing queues ahead) ----
            def _x_src(it):
                xt = xs_pool.tile([P, C], F32, tag="xt")
                nc.sync.dma_start(out=xt[:], in_=x_d[it * P : (it + 1) * P, :])
                return xt

            with nc.named_scope("ln1"), \
                    tc.tile_pool(name="ps_m1", bufs=2, space="PSUM") as ps_m1:
                layernorm_to_uT(_x_src, uT, ps_m1)

            # qkv weights arrive while LN1 runs
            for k in range(KC):
                nc.sync.dma_start(out=wq_t[k][:], in_=wqT_d[k * P : (k + 1) * P, :])
            for k in range(KC):
                nc.sync.dma_start(out=wk_t[k][:], in_=wkT_d[k * P : (k + 1) * P, :])
            for k in range(KC):
                nc.sync.dma_start(out=wv_t[k][:], in_=wvT_d[k * P : (k + 1) * P, :])
            for mk, dstA in ((qm_d, qA), (km_d, kA)):
                map_ = mk[:]
                src_b = bass.AP(
                    tensor=map_.tensor, offset=map_.offset,
                    ap=[map_.ap[0], [0, H // 2], map_.ap[1]],
                )
                ev = dstA[:].rearrange("p (h n) -> p h n", h=H)
                nc.sync.dma_start(out=ev[HD:P, 0:H:2, :], in_=src_b)
                nc.sync.dma_start(out=ev[0:HD, 1:H:2, :], in_=src_b)
            m1b_t = persist.tile([P, FH], F32, tag="m1b")
            nc.sync.dma_start(out=m1b_t[:], in_=m1b_d.rearrange("(f p) -> p f", p=P))
            if with_biases:
                qb_t = persist.tile([P, KC], F32, tag="qb")
                nc.sync.dma_start(
                    out=qb_t[:], in_=qb_d.rearrange("(c p) -> p c", p=P)
                )
                kb_t = persist.tile([P, KC], F32, tag="kb")
                nc.sync.dma_start(
                    out=kb_t[:], in_=kb_d.rearrange("(c p) -> p c", p=P)
                )
                vb_r = persist.tile([1, C], BF, tag="vb_r")
                nc.sync.dma_start(out=vb_r[:], in_=vb_d.rearrange("(a c) -> a c", a=1))
                pb_r = persist.tile([1, C], BF, tag="pb_r")
                nc.sync.dma_start(out=pb_r[:], in_=pb_d.rearrange("(a c) -> a c", a=1))
                m2b_r = persist.tile([1, C], BF, tag="m2b_r")
                nc.sync.dma_start(
                    out=m2b_r[:], in_=m2b_d.rearrange("(a c) -> a c", a=1)
                )

            # ---- phase 2: QKV ----
            # q/k feature-major into augmented head tiles (partitions 0:64):
            # q copies on ACT, k copies on DVE to balance engine load.
            with nc.named_scope("qkv"), \
                    tc.tile_pool(name="ps_qkv", bufs=6, space="PSUM") as ps_acc:
                for which, w_t, dstA in (("q", wq_t, qA), ("k", wk_t, kA)):
                    for co in range(KC):  # output-feature tile: heads 2co, 2co+1
                        for nch in range(2):  # 512-token chunks
                            ps = ps_acc.tile([P, C], F32, tag="acc")
                            for k in range(KC):
                                nc.tensor.matmul(
                                    ps[:],
                                    w_t[k][:, co * P : (co + 1) * P],
                                    uT[:, k, nch * 512 : (nch + 1) * 512],
                                    start=(k == 0), stop=(k == KC - 1),
                                )
                            for half in range(2):
                                h = 2 * co + half
                                pr = slice(half * HD, (half + 1) * HD)
                                dst = dstA[pr, h * N + nch * 512 : h * N + (nch + 1) * 512]
                                src = ps[pr, :]
                                if which == "q":
                                    if with_biases:
                                        nc.scalar.activation(
                                            out=dst, in_=src,
                                            func=mybir.ActivationFunctionType.Identity,
                                            bias=qb_t[half * HD : (half + 1) * HD, co : co + 1],
                                            scale=1.0,
                                        )
                                    else:
                                        nc.scalar.copy(out=dst, in_=src)
                                else:
                                    if with_biases:
                                        nc.vector.tensor_scalar_add(
                                            dst, src,
                                            kb_t[half * HD : (half + 1) * HD, co : co + 1],
                                        )
                                    elif (co + nch) % 2 == 0:
                                        nc.vector.tensor_copy(out=dst, in_=src)
                                    else:
                                        nc.scalar.copy(out=dst, in_=src)
                # v token-major packed into vext (+ ones column for denominators)
                for it in range(NT):
                    ps = ps_acc.tile([P, C], F32, tag="acc")
                    for k in range(KC):
                        nc.tensor.matmul(
                            ps[:],
                            uT[:, k, it * P : (it + 1) * P],
                            wv_t[k][:],
                            start=(k == 0),
                            stop=(k == KC - 1) and not with_biases,
                        )
                    if with_biases:
                        nc.tensor.matmul(
                            ps[:], ones_row[:], vb_r[:], start=False, stop=True
                        )
                    vx = vext[it]
                    nc.gpsimd.memset(vx[:, :, HD : HD + 1], 1.0)
                    nc.vector.tensor_copy(
                        out=vx[:, :, 0:HD],
                        in_=ps[:].rearrange("p (h d) -> p h d", h=H),
                    )

            # proj weights arrive during QKV/attention
            for k in range(KC):
                nc.sync.dma_start(out=wp_t[k][:], in_=wpT_d[k * P : (k + 1) * P, :])

            # ---- phase 3+4: per-head banded scores/exp then AV ----
            # score tiles for j-tile pairs (2jp, 2jp+1) share one PSUM tile:
            # pair widths are <=512 so both sub-tiles fit one PSUM bank and
            # exp runs as one wide activation per pair.
            pair_off = {}
            for jt in range(NT):
                i0, i1 = _band_range(jt)
                pair_off[jt] = 0 if jt % 2 == 0 else (
                    _band_range(jt - 1)[1] - _band_range(jt - 1)[0]
                )
            with nc.named_scope("attn"), \
                    tc.tile_pool(name="ps_sc", bufs=2, space="PSUM") as ps_s, \
                    tc.tile_pool(name="ps_po", bufs=4, space="PSUM") as ps_m:
                # quad offsets: jts 4g..4g+3 share one 2-bank psum tile; subs
                # 0,1 pack into bank0 and 2,3 into bank1 (a matmul output
                # must stay within one PSUM bank).
                qoff = {}
                for jt in range(NT):
                    g, sub = jt // 4, jt % 4
                    if sub == 0:
                        qoff[jt] = 0
                    elif sub == 1:
                        qoff[jt] = _band_range(jt - 1)[1] - _band_range(jt - 1)[0]
                    elif sub == 2:
                        qoff[jt] = 512
                    else:
                        qoff[jt] = 512 + _band_range(jt - 1)[1] - _band_range(jt - 1)[0]
                for h in range(H):
                    e_tiles = {}
                    for g in range(NT // 4):
                        jlast = 4 * g + 3
                        wtot = qoff[jlast] + _band_range(jlast)[1] - _band_range(jlast)[0]
                        ps = ps_s.tile([P, 1024], F32, tag="sc")
                        for sub in range(4):
                            jt = 4 * g + sub
                            i0, i1 = _band_range(jt)
                            nc.tensor.matmul(
                                ps[:, qoff[jt] : qoff[jt] + (i1 - i0)],
                                kA[:, h * N + jt * P : h * N + (jt + 1) * P],
                                qA[:, h * N + i0 : h * N + i1],
                                start=True, stop=True,
                            )
                        et = e_pool.tile([P, 1024], BF, tag="et")
                        nc.scalar.activation(
                            out=et[:, 0:wtot], in_=ps[:, 0:wtot],
                            func=mybir.ActivationFunctionType.Exp,
                            bias=nbig_t[:], scale=SCALE,
                        )
                        e_tiles[g] = et
                    for it in range(NT):
                        po = ps_m.tile([P, HD + 1], F32, tag="misc")
                        # center tile zero-initializes the whole psum region;
                        # the +-64 edge pieces accumulate partial partitions.
                        c_off = qoff[it] + it * P - _band_range(it)[0]
                        nc.tensor.matmul(
                            po[:],
                            e_tiles[it // 4][:, c_off : c_off + P],
                            vext[it][:, h, :],
                            start=True, stop=False,
                        )
                        pieces = []
                        if it > 0:
                            jt = it - 1
                            off = qoff[jt] + it * P - _band_range(jt)[0]
                            pieces.append((jt, off, 0))      # po[0:64]
                        if it < NT - 1:
                            jt = it + 1
                            off = qoff[jt]                   # i starts at its i0
                            pieces.append((jt, off, HD))     # po[64:128]
                        for n_, (jt, off, pbase) in enumerate(pieces):
                            nc.tensor.matmul(
                                po[pbase : pbase + HD, :],
                                e_tiles[jt // 4][:, off : off + HD],
                                vext[jt][:, h, :],
                                start=False, stop=(n_ == len(pieces) - 1),
                            )
                        # D >= ~0.4 always, so the reference's +1e-6 in the
                        # denominator is far below bf16 noise; skip it.
                        r = r_pool.tile([P, 1], F32, tag="r")
                        nc.vector.reciprocal(r[:], po[:, HD : HD + 1])
                        nc.vector.tensor_scalar_mul(
                            O_sb[it][:, h * HD : (h + 1) * HD], po[:, 0:HD], r[:]
                        )

            # mlp weights arrive during attention
            for k in range(KC):
                nc.sync.dma_start(out=w1_t[k][:], in_=w1T_d[k * P : (k + 1) * P, :])
            for k in range(FH):
                nc.sync.dma_start(out=w2_t[k][:], in_=w2T_d[k * P : (k + 1) * P, :])

            # ---- phase 5: transpose O + proj + residual ----
            ps_m2 = ec(tc.tile_pool(name="ps_m2", bufs=2, space="PSUM"))
            ps_acc = ec(tc.tile_pool(name="ps_acc2", bufs=5, space="PSUM"))
            ps_m = ps_m2
            with nc.named_scope("proj"):
                for it in range(NT):
                    pt = ps_m.tile([P, KC, P], BF, tag="misc")
                    for k in range(KC):
                        nc.tensor.transpose(
                            pt[:, k, :], O_sb[it][:, k * P : (k + 1) * P], ident[:]
                        )
                    nc.scalar.copy(out=OT[:, :, it * P : (it + 1) * P], in_=pt[:])

                for it in range(NT):
                    ps = ps_acc.tile([P, C], F32, tag="acc")
                    for k in range(KC):
                        nc.tensor.matmul(
                            ps[:],
                            OT[:, k, it * P : (it + 1) * P],
                            wp_t[k][:],
                            start=(k == 0),
                            stop=(k == KC - 1) and not with_biases,
                        )
                    if with_biases:
                        nc.tensor.matmul(
                            ps[:], ones_row[:], pb_r[:], start=False, stop=True
                        )
                    xr = xr_pool.tile([P, C], F32, tag="xr")
                    nc.sync.dma_start(out=xr[:], in_=x_d[it * P : (it + 1) * P, :])
                    nc.vector.tensor_add(x1[it][:], xr[:], ps[:])

            # ---- phase 7: LN2 (reuses uT tiles) ----
            with nc.named_scope("ln2"):
                layernorm_to_uT(lambda it: x1[it], uT, ps_m2)

            # ---- phase 8+9: MLP, chunk-interleaved so MLP2 of chunk 0
            # overlaps MLP1 of chunk 1 ----
            def mlp1_chunk(nch):
                for fh in range(FH):
                    ps = ps_acc.tile([P, C], F32, tag="acc")
                    for k in range(KC):
                        nc.tensor.matmul(
                            ps[:],
                            w1_t[k][:, fh * P : (fh + 1) * P],
                            uT[:, k, nch * 512 : (nch + 1) * 512],
                            start=(k == 0), stop=(k == KC - 1),
                        )
                    nc.scalar.activation(
                        out=h1T[fh][:, nch * 512 : (nch + 1) * 512], in_=ps[:],
                        func=mybir.ActivationFunctionType.Gelu,
                        bias=m1b_t[:, fh : fh + 1], scale=1.0,
                    )

            def mlp2_tiles(its):
                for it in its:
                    ps = ps_acc.tile([P, C], F32, tag="acc")
                    for fh in range(FH):
                        nc.tensor.matmul(
                            ps[:],
                            h1T[fh][:, it * P : (it + 1) * P],
                            w2_t[fh][:],
                            start=(fh == 0),
                            stop=(fh == FH - 1) and not with_biases,
                        )
                    if with_biases:
                        nc.tensor.matmul(
                            ps[:], ones_row[:], m2b_r[:], start=False, stop=True
                        )
                    ot = out_pool.tile([P, C], F32, tag="ot")
                    nc.vector.tensor_add(ot[:], x1[it][:], ps[:])
                    nc.sync.dma_start(out=y_d[it * P : (it + 1) * P, :], in_=ot[:])

            with nc.named_scope("mlp"):
                mlp1_chunk(0)
                mlp2_tiles(range(0, 4))
                mlp1_chunk(1)
                mlp2_tiles(range(4, 8))

    _split_excess_waits(nc)
    return nc


_PROGRAMS = {}


def _get_program(with_biases: bool):
    if with_biases not in _PROGRAMS:
        _PROGRAMS[with_biases] = _build_program(with_biases)
    return _PROGRAMS[with_biases]


def _gelu_exact(x):
    from math import sqrt

    from numpy import vectorize  # noqa: F401  (erf via scipy-free path below)

    # exact erf-based gelu in float64
    import numpy as _np

    # erf via numpy's math: use vectorized math.erf for accuracy
    from math import erf

    ex = _np.vectorize(erf)(x / sqrt(2.0))
    return 0.5 * x * (1.0 + ex)


def _reference_np(x_token, wq, wk, wv, w_proj, b_proj, g1, b1, g2, b2,
                  w1, bb1, w2, bb2, idx):
    """float64 numpy fallback (used only if the banded-kernel preconditions
    fail, e.g. a cluster larger than MAX_CLUSTER tokens)."""
    x = x_token.astype(np.float64)
    out = np.empty_like(x)
    scale = HD ** -0.5
    for b in range(x.shape[0]):
        xb = x[b]
        mu = xb.mean(-1, keepdims=True)
        var = ((xb - mu) ** 2).mean(-1, keepdims=True)
        t = (xb - mu) / np.sqrt(var + LN_EPS) * g1 + b1
        q = (t @ wq.T).reshape(N, H, HD).transpose(1, 0, 2)
        k = (t @ wk.T).reshape(N, H, HD).transpose(1, 0, 2)
        v = (t @ wv.T).reshape(N, H, HD).transpose(1, 0, 2)
        s = np.einsum("hid,hjd->hij", q, k) * scale
        same = idx[b][None, :, None] == idx[b][None, None, :]
        e = np.exp(s) * same
        attn = (e + ATT_EPS / N) / (e.sum(-1, keepdims=True) + ATT_EPS)
        o = np.einsum("hij,hjd->hid", attn, v)
        o = o.transpose(1, 0, 2).reshape(N, C) @ w_proj.T + b_proj
        xr = xb + o
        mu = xr.mean(-1, keepdims=True)
        var = ((xr - mu) ** 2).mean(-1, keepdims=True)
        hh = (xr - mu) / np.sqrt(var + LN_EPS) * g2 + b2
        m = _gelu_exact(hh @ w1.T + bb1) @ w2.T + bb2
        out[b] = xr + m
    return out.astype(np.float32)


def kernel(**inputs):
    x_token = np.ascontiguousarray(np.asarray(inputs["x_token"], np.float32))
    idx = np.asarray(inputs["idx_cluster"]).astype(np.int64)
    wq = np.asarray(inputs["wq"], np.float32)
    wk = np.asarray(inputs["wk"], np.float32)
    wv = np.asarray(inputs["wv"], np.float32)
    w_proj = np.asarray(inputs["w_proj"], np.float32)
    b_proj = np.asarray(inputs["b_proj"], np.float32)
    g1 = np.asarray(inputs["g1"], np.float32)
    b1 = np.asarray(inputs["b1"], np.float32)
    g2 = np.asarray(inputs["g2"], np.float32)
    b2 = np.asarray(inputs["b2"], np.float32)
    w1 = np.asarray(inputs["w1"], np.float32)
    bb1 = np.asarray(inputs["bb1"], np.float32)
    w2 = np.asarray(inputs["w2"], np.float32)
    bb2 = np.asarray(inputs["bb2"], np.float32)

    perms = []
    ok = idx.min() >= 0 and idx.max() < CLN
    if ok:
        for b in range(B):
            sizes = np.bincount(idx[b], minlength=CLN)
            if sizes.max() > MAX_CLUSTER:
                ok = False
                break
    if not ok:
        return _reference_np(x_token, wq, wk, wv, w_proj, b_proj, g1, b1,
                             g2, b2, w1, bb1, w2, bb2, idx)

    # fold LN gains/biases into adjacent weights
    wqT = (g1[:, None] * wq.T).astype(BF16)
    wkT = (g1[:, None] * wk.T).astype(BF16)
    wvT = (g1[:, None] * wv.T).astype(BF16)
    wpT = np.ascontiguousarray(w_proj.T).astype(BF16)
    w1T = (g2[:, None] * w1.T).astype(BF16)
    w2T = np.ascontiguousarray(w2.T).astype(BF16)
    qb = (wq @ b1).astype(np.float32)
    kb = (wk @ b1).astype(np.float32)
    vb = (wv @ b1).astype(BF16)
    pb = b_proj.astype(BF16)
    m1b = (bb1 + w1 @ b2).astype(np.float32)
    m2b = bb2.astype(BF16)

    # g1/g2/bb1/b2 fold into weights and the gelu bias (m1b), so any values
    # are handled by the fast path. Nonzero b1/b_proj/bb2 would need the
    # untested bias program variant -- setup_inputs() hardcodes them to
    # zero, so route that (unreachable) case to the numpy fallback instead.
    if np.any(qb) or np.any(kb) or np.any(vb) or np.any(pb) or np.any(m2b):
        return _reference_np(x_token, wq, wk, wv, w_proj, b_proj, g1, b1,
                             g2, b2, w1, bb1, w2, bb2, idx)
    with_biases = False

    shared = dict(
        wqT=wqT, wkT=wkT, wvT=wvT, wpT=wpT, w1T=w1T, w2T=w2T, m1b=m1b,
    )

    in_maps = []
    ar = np.arange(CLN)
    for b in range(B):
        perm = np.argsort(idx[b], kind="stable")
        perms.append(perm)
        cid = idx[b][perm]
        onehot = (cid[None, :] == ar[:, None])
        in_maps.append(
            dict(
                shared,
                x=np.ascontiguousarray(x_token[b][perm]),
                qmask=(ALPHA_Q * onehot).astype(BF16),
                kmask=(ALPHA_K * onehot).astype(BF16),
            )
        )

    nc = _get_program(with_biases)
    res = run_bass_kernel_spmd(nc, in_maps, list(range(B)))
    global LAST_RESULTS
    LAST_RESULTS = res
    out = np.empty((B, N, C), np.float32)
    for b in range(B):
        out[b][perms[b]] = res.results[b]["y"]
    return out


LAST_RESULTS = None


# revision 69
# speedup vs baseline: 1.0283x; 1.0193x over previous
"""Trainium2 Bass kernel for nn_CBlock2 (sparse cluster attention block).

Strategy: data-parallel over batch B=8 across 8 cores. Per core, tokens are
host-sorted by cluster id so same-cluster attention pairs lie in a 3-tile
band around the diagonal; the cluster mask is folded into the score matmul
via one-hot augmentation rows (exp(s - 50) == 0 for cross-cluster pairs).
LayerNorm gains/biases are folded into the adjacent weight matrices on the
host. All matmuls run in bf16 with fp32 PSUM accumulation; the residual
stream stays fp32.
"""
import sys

sys.path.insert(0, "/opt/trn_rl_repo")

import numpy as np
import ml_dtypes

import concourse.bass as bass
import concourse.mybir as mybir
import concourse.tile as tile
from concourse.bass_utils import run_bass_kernel_spmd
from concourse.masks import make_identity

BF16 = ml_dtypes.bfloat16

B, N, C, H, PD, CLN = 8, 1024, 512, 8, 256, 64
HD = C // H          # 64
HID = 4 * C          # 2048
LN_EPS = 1e-5
ATT_EPS = 1e-6
P = 128
NT = N // P          # 8 token tiles
KC = C // P          # 4 contraction tiles over C
FH = HID // P        # 16 hidden tiles
SCALE = HD ** -0.5   # 0.125
ALPHA_Q = 16.0
ALPHA_K = 25.0
BIG = ALPHA_Q * ALPHA_K * SCALE  # 50.0: mask margin inside exp
MAX_CLUSTER = 65     # band bound: all same-cluster pairs within +-64 tokens

F32 = mybir.dt.float32
BF = mybir.dt.bfloat16


def _split_excess_waits(nc, max_waits=1):
    """walrus in this env rejects >1 sync-wait on one instruction; hoist
    excess waits onto same-engine no-op carriers inserted just before."""
    for f in nc.m.functions:
        for bb in f.blocks:
            new_insts = []
            for inst in bb.instructions:
                si = inst.sync_info
                if si is not None and si.on_wait and len(si.on_wait) > max_waits:
                    waits = list(si.on_wait)
                    excess, keep = waits[:-max_waits], waits[-max_waits:]
                    for ci in range(0, len(excess), max_waits):
                        chunk = excess[ci : ci + max_waits]
                        new_insts.append(
                            mybir.InstNoOp(
                                name=f"{inst.name}-ws{ci}",
                                engine=inst.engine,
                                ins=[],
                                outs=[],
                                sync_info=mybir.SyncInfo(on_wait=chunk, on_update=[]),
                            )
                        )
                    inst.sync_info = mybir.SyncInfo(
                        on_wait=keep, on_update=list(si.on_update)
                    )
                new_insts.append(inst)
            bb.instructions = new_insts


_PROGRAM = None


def _band_range(jt):
    """i-range covered by the score tile of j-tile jt (+-64-token band)."""
    i0 = max(0, jt * P - 64)
    i1 = min(N, (jt + 1) * P + 64)
    return i0, i1


def _build_program(with_biases: bool):
    """Build the per-core SPMD program. with_biases=False omits the bias
    paths (all reference biases are zero in the standard setup)."""
    nc = bass.Bass()

    x_d = nc.declare_dram_parameter("x", [N, C], F32, isOutput=False)
    wqT_d = nc.declare_dram_parameter("wqT", [C, C], BF, isOutput=False)
    wkT_d = nc.declare_dram_parameter("wkT", [C, C], BF, isOutput=False)
    wvT_d = nc.declare_dram_parameter("wvT", [C, C], BF, isOutput=False)
    wpT_d = nc.declare_dram_parameter("wpT", [C, C], BF, isOutput=False)
    w1T_d = nc.declare_dram_parameter("w1T", [C, HID], BF, isOutput=False)
    w2T_d = nc.declare_dram_parameter("w2T", [HID, C], BF, isOutput=False)
    m1b_d = nc.declare_dram_parameter("m1b", [HID], F32, isOutput=False)
    qm_d = nc.declare_dram_parameter("qmask", [CLN, N], BF, isOutput=False)
    km_d = nc.declare_dram_parameter("kmask", [CLN, N], BF, isOutput=False)
    if with_biases:
        qb_d = nc.declare_dram_parameter("qb", [C], F32, isOutput=False)
        kb_d = nc.declare_dram_parameter("kb", [C], F32, isOutput=False)
        vb_d = nc.declare_dram_parameter("vb", [C], BF, isOutput=False)
        pb_d = nc.declare_dram_parameter("pb", [C], BF, isOutput=False)
        m2b_d = nc.declare_dram_parameter("m2b", [C], BF, isOutput=False)
    y_d = nc.declare_dram_parameter("y", [N, C], F32, isOutput=True)

    with tile.TileContext(nc) as tc:
        from contextlib import ExitStack

        with ExitStack() as ctx:
            ec = ctx.enter_context
            persist = ec(tc.tile_pool(name="persist", bufs=1))
            w_small = ec(tc.tile_pool(name="w_small", bufs=1))
            w_big = ec(tc.tile_pool(name="w_big", bufs=1))
            xs_pool = ec(tc.tile_pool(name="xs", bufs=8))
            xr_pool = ec(tc.tile_pool(name="xr", bufs=3))
            ln_pool = ec(tc.tile_pool(name="ln", bufs=6))
            e_pool = ec(tc.tile_pool(name="epool", bufs=6))
            r_pool = ec(tc.tile_pool(name="rpool", bufs=8))
            out_pool = ec(tc.tile_pool(name="outp", bufs=3))

            # ---- tiny constants (cheap, emitted first) ----
            ident = persist.tile([P, P], BF, tag="ident")
            make_identity(nc, ident[:])
            eps_t = persist.tile([P, 1], F32, tag="eps_t")
            nc.vector.memset(eps_t[:], LN_EPS)
            nbig_t = persist.tile([P, 1], F32, tag="nbig_t")
            nc.vector.memset(nbig_t[:], -BIG)
            if with_biases:
                ones_row = persist.tile([1, P], BF, tag="ones_row")
                nc.vector.memset(ones_row[:], 1.0)

            # ---- persistent activations ----
            # qA/kA: one wide tile, head h at free cols [h*N,(h+1)*N). Even
            # heads keep q rows on partitions 0:64 + mask rows on 64:128;
            # odd heads are flipped so PSUM->SBUF copies stay
            # partition-aligned.
            qA = persist.tile([P, H * N], BF, tag="qA")
            kA = persist.tile([P, H * N], BF, tag="kA")
            vext = [persist.tile([P, H, HD + 1], BF, tag=f"vx{t}", name=f"vx{t}") for t in range(NT)]
            uT = persist.tile([P, KC, N], BF, tag="uT")
            OT = persist.tile([P, KC, N], BF, tag="OT")
            O_sb = [persist.tile([P, C], BF, tag=f"Osb{t}", name=f"Osb{t}") for t in range(NT)]
            x1 = [persist.tile([P, C], F32, tag=f"x1{t}", name=f"x1{t}") for t in range(NT)]
            h1T = [persist.tile([P, N], BF, tag=f"h1T{k}", name=f"h1T{k}") for k in range(FH)]

            wq_t = [w_small.tile([P, C], BF, tag=f"wq{k}", name=f"wq{k}") for k in range(KC)]
            wk_t = [w_small.tile([P, C], BF, tag=f"wk{k}", name=f"wk{k}") for k in range(KC)]
            wv_t = [w_small.tile([P, C], BF, tag=f"wv{k}", name=f"wv{k}") for k in range(KC)]
            wp_t = [w_small.tile([P, C], BF, tag=f"wp{k}", name=f"wp{k}") for k in range(KC)]
            w2_t = [w_small.tile([P, C], BF, tag=f"w2{k}", name=f"w2{k}") for k in range(FH)]
            w1_t = [w_big.tile([P, HID], BF, tag=f"w1{k}", name=f"w1{k}") for k in range(KC)]

            def layernorm_to_uT(src_tile_fn, dst_uT, ps_m):
                """token-major f32 tiles -> normalized bf16, PE-transposed into
                feature-major dst_uT (KC tiles of [P, N])."""
                for it in range(NT):
                    xt = src_tile_fn(it)
                    stats = ln_pool.tile([P, 6], F32, tag="stats")
                    nc.vector.bn_stats(out=stats[:], in_=xt[:])
                    mv = ln_pool.tile([P, 2], F32, tag="mv")
                    nc.vector.bn_aggr(out=mv[:], in_=stats[:])
                    std = ln_pool.tile([P, 1], F32, tag="std")
                    nc.scalar.activation(
                        out=std[:], in_=mv[:, 1:2],
                        func=mybir.ActivationFunctionType.Sqrt,
                        bias=eps_t[:], scale=1.0,
                    )
                    nc.vector.reciprocal(out=std[:], in_=std[:])
                    u = ln_pool.tile([P, C], BF, tag="u")
                    nc.vector.tensor_scalar(
                        out=u[:], in0=xt[:],
                        scalar1=mv[:, 0:1], scalar2=std[:],
                        op0=mybir.AluOpType.subtract, op1=mybir.AluOpType.mult,
                    )
                    pt = ps_m.tile([P, KC, P], BF, tag="misc")
                    for k in range(KC):
                        nc.tensor.transpose(
                            pt[:, k, :], u[:, k * P : (k + 1) * P], ident[:]
                        )
                    nc.scalar.copy(
                        out=dst_uT[:, :, it * P : (it + 1) * P], in_=pt[:]
                    )

            # ---- phase 1: LN1 (x streamed in first -- nothing queues ahead) ----
            def _x_src(it):
                xt = xs_pool.tile([P, C], F32, tag="xt")
                nc.sync.dma_start(out=xt[:], in_=x_d[it * P : (it + 1) * P, :])
                return xt

            with nc.named_scope("ln1"), \
                    tc.tile_pool(name="ps_m1", bufs=2, space="PSUM") as ps_m1:
                layernorm_to_uT(_x_src, uT, ps_m1)

            # qkv weights arrive while LN1 runs
            for k in range(KC):
                nc.sync.dma_start(out=wq_t[k][:], in_=wqT_d[k * P : (k + 1) * P, :])
            for k in range(KC):
                nc.sync.dma_start(out=wk_t[k][:], in_=wkT_d[k * P : (k + 1) * P, :])
            for k in range(KC):
                nc.sync.dma_start(out=wv_t[k][:], in_=wvT_d[k * P : (k + 1) * P, :])
            for mk, dstA in ((qm_d, qA), (km_d, kA)):
                map_ = mk[:]
                src_b = bass.AP(
                    tensor=map_.tensor, offset=map_.offset,
                    ap=[map_.ap[0], [0, H // 2], map_.ap[1]],
                )
                ev = dstA[:].rearrange("p (h n) -> p h n", h=H)
                nc.sync.dma_start(out=ev[HD:P, 0:H:2, :], in_=src_b)
                nc.sync.dma_start(out=ev[0:HD, 1:H:2, :], in_=src_b)
            m1b_t = persist.tile([P, FH], F32, tag="m1b")
            nc.sync.dma_start(out=m1b_t[:], in_=m1b_d.rearrange("(f p) -> p f", p=P))
            if with_biases:
                qb_t = persist.tile([P, KC], F32, tag="qb")
                nc.sync.dma_start(
                    out=qb_t[:], in_=qb_d.rearrange("(c p) -> p c", p=P)
                )
                kb_t = persist.tile([P, KC], F32, tag="kb")
                nc.sync.dma_start(
                    out=kb_t[:], in_=kb_d.rearrange("(c p) -> p c", p=P)
                )
                vb_r = persist.tile([1, C], BF, tag="vb_r")
                nc.sync.dma_start(out=vb_r[:], in_=vb_d.rearrange("(a c) -> a c", a=1))
                pb_r = persist.tile([1, C], BF, tag="pb_r")
                nc.sync.dma_start(out=pb_r[:], in_=pb_d.rearrange("(a c) -> a c", a=1))
                m2b_r = persist.tile([1, C], BF, tag="m2b_r")
                nc.sync.dma_start(
                    out=m2b_r[:], in_=m2b_d.rearrange("(a c) -> a c", a=1)
                )

            # ---- phase 2: QKV ----
            # q/k feature-major into augmented head tiles (partitions 0:64):
            # q copies on ACT, k copies on DVE to balance engine load.
            with nc.named_scope("qkv"), \
                    tc.tile_pool(name="ps_qkv", bufs=6, space="PSUM") as ps_acc:
                for which, w_t, dstA in (("q", wq_t, qA), ("k", wk_t, kA)):
                    for co in range(KC):  # output-feature tile: heads 2co, 2co+1
                        for nch in range(2):  # 512-token chunks
                            ps = ps_acc.tile([P, C], F32, tag="acc")
                            for k in range(KC):
                                nc.tensor.matmul(
                                    ps[:],
                                    w_t[k][:, co * P : (co + 1) * P],
                                    uT[:, k, nch * 512 : (nch + 1) * 512],
                                    start=(k == 0), stop=(k == KC - 1),
                                )
                            for half in range(2):
                                h = 2 * co + half
                                pr = slice(half * HD, (half + 1) * HD)
                                dst = dstA[pr, h * N + nch * 512 : h * N + (nch + 1) * 512]
                                src = ps[pr, :]
                                if which == "q":
                                    if with_biases:
                                        nc.scalar.activation(
                                            out=dst, in_=src,
                                            func=mybir.ActivationFunctionType.Identity,
                                            bias=qb_t[half * HD : (half + 1) * HD, co : co + 1],
                                            scale=1.0,
                                        )
                                    else:
                                        nc.scalar.copy(out=dst, in_=src)
                                else:
                                    if with_biases:
                                        nc.vector.tensor_scalar_add(
                                            dst, src,
                                            kb_t[half * HD : (half + 1) * HD, co : co + 1],
                                        )
                                    elif (co + nch) % 2 == 0:
                                        nc.vector.tensor_copy(out=dst, in_=src)
                                    else:
                                        nc.scalar.copy(out=dst, in_=src)
                # v token-major packed into vext (+ ones column for denominators)
                for it in range(NT):
                    ps = ps_acc.tile([P, C], F32, tag="acc")
                    for k in range(KC):
                        nc.tensor.matmul(
                            ps[:],
                            uT[:, k, it * P : (it + 1) * P],
                            wv_t[k][:],
                            start=(k == 0),
                            stop=(k == KC - 1) and not with_biases,
                        )
                    if with_biases:
                        nc.tensor.matmul(
                            ps[:], ones_row[:], vb_r[:], start=False, stop=True
                        )
                    vx = vext[it]
                    nc.gpsimd.memset(vx[:, :, HD : HD + 1], 1.0)
                    nc.vector.tensor_copy(
                        out=vx[:, :, 0:HD],
                        in_=ps[:].rearrange("p (h d) -> p h d", h=H),
                    )

            # proj weights arrive during QKV/attention
            for k in range(KC):
                nc.sync.dma_start(out=wp_t[k][:], in_=wpT_d[k * P : (k + 1) * P, :])

            # ---- phase 3+4: per-head banded scores/exp then AV ----
            # score tiles for j-tile pairs (2jp, 2jp+1) share one PSUM tile:
            # pair widths are <=512 so both sub-tiles fit one PSUM bank and
            # exp runs as one wide activation per pair.
            pair_off = {}
            for jt in range(NT):
                i0, i1 = _band_range(jt)
                pair_off[jt] = 0 if jt % 2 == 0 else (
                    _band_range(jt - 1)[1] - _band_range(jt - 1)[0]
                )
            with nc.named_scope("attn"), \
                    tc.tile_pool(name="ps_sc", bufs=2, space="PSUM") as ps_s, \
                    tc.tile_pool(name="ps_po", bufs=4, space="PSUM") as ps_m:
                # quad offsets: jts 4g..4g+3 share one 2-bank psum tile; subs
                # 0,1 pack into bank0 and 2,3 into bank1 (a matmul output
                # must stay within one PSUM bank).
                qoff = {}
                for jt in range(NT):
                    g, sub = jt // 4, jt % 4
                    if sub == 0:
                        qoff[jt] = 0
                    elif sub == 1:
                        qoff[jt] = _band_range(jt - 1)[1] - _band_range(jt - 1)[0]
                    elif sub == 2:
                        qoff[jt] = 512
                    else:
                        qoff[jt] = 512 + _band_range(jt - 1)[1] - _band_range(jt - 1)[0]
                for h in range(H):
                    e_tiles = {}
                    for g in range(NT // 4):
                        jlast = 4 * g + 3
                        wtot = qoff[jlast] + _band_range(jlast)[1] - _band_range(jlast)[0]
                        ps = ps_s.tile([P, 1024], F32, tag="sc")
                        for sub in range(4):
                            jt = 4 * g + sub
                            i0, i1 = _band_range(jt)
                            nc.tensor.matmul(
                                ps[:, qoff[jt] : qoff[jt] + (i1 - i0)],
                                kA[:, h * N + jt * P : h * N + (jt + 1) * P],
                                qA[:, h * N + i0 : h * N + i1],
                                start=True, stop=True,
                            )
                        et = e_pool.tile([P, 1024], BF, tag="et")
                        nc.scalar.activation(
                            out=et[:, 0:wtot], in_=ps[:, 0:wtot],
                            func=mybir.ActivationFunctionType.Exp,
                            bias=nbig_t[:], scale=SCALE,
                        )
                        e_tiles[g] = et
                    for it in range(NT):
                        po = ps_m.tile([P, HD + 1], F32, tag="misc")
                        # center tile zero-initializes the whole psum region;
                        # the +-64 edge pieces accumulate partial partitions.
                        c_off = qoff[it] + it * P - _band_range(it)[0]
                        nc.tensor.matmul(
                            po[:],
                            e_tiles[it // 4][:, c_off : c_off + P],
                            vext[it][:, h, :],
                            start=True, stop=False,
                        )
                        pieces = []
                        if it > 0:
                            jt = it - 1
                            off = qoff[jt] + it * P - _band_range(jt)[0]
                            pieces.append((jt, off, 0))      # po[0:64]
                        if it < NT - 1:
                            jt = it + 1
                            off = qoff[jt]                   # i starts at its i0
                            pieces.append((jt, off, HD))     # po[64:128]
                        for n_, (jt, off, pbase) in enumerate(pieces):
                            nc.tensor.matmul(
                                po[pbase : pbase + HD, :],
                                e_tiles[jt // 4][:, off : off + HD],
                                vext[jt][:, h, :],
                                start=False, stop=(n_ == len(pieces) - 1),
                            )
                        # D >= ~0.4 always, so the reference's +1e-6 in the
                        # denominator is far below bf16 noise; skip it.
                        r = r_pool.tile([P, 1], F32, tag="r")
                        nc.vector.reciprocal(r[:], po[:, HD : HD + 1])
                        nc.vector.tensor_scalar_mul(
                            O_sb[it][:, h * HD : (h + 1) * HD], po[:, 0:HD], r[:]
                        )

            # mlp weights arrive during attention
            for k in range(KC):
                nc.sync.dma_start(out=w1_t[k][:], in_=w1T_d[k * P : (k + 1) * P, :])
            for k in range(FH):
                nc.sync.dma_start(out=w2_t[k][:], in_=w2T_d[k * P : (k + 1) * P, :])

            # ---- phase 5: transpose O + proj + residual ----
            ps_m2 = ec(tc.tile_pool(name="ps_m2", bufs=2, space="PSUM"))
            ps_acc = ec(tc.tile_pool(name="ps_acc2", bufs=5, space="PSUM"))
            ps_m = ps_m2
            with nc.named_scope("proj"):
                for it in range(NT):
                    pt = ps_m.tile([P, KC, P], BF, tag="misc")
                    for k in range(KC):
                        nc.tensor.transpose(
                            pt[:, k, :], O_sb[it][:, k * P : (k + 1) * P], ident[:]
                        )
                    nc.scalar.copy(out=OT[:, :, it * P : (it + 1) * P], in_=pt[:])

                for it in range(NT):
                    ps = ps_acc.tile([P, C], F32, tag="acc")
                    for k in range(KC):
                        nc.tensor.matmul(
                            ps[:],
                            OT[:, k, it * P : (it + 1) * P],
                            wp_t[k][:],
                            start=(k == 0),
                            stop=(k == KC - 1) and not with_biases,
                        )
                    if with_biases:
                        nc.tensor.matmul(
                            ps[:], ones_row[:], pb_r[:], start=False, stop=True
                        )
                    xr = xr_pool.tile([P, C], F32, tag="xr")
                    nc.sync.dma_start(out=xr[:], in_=x_d[it * P : (it + 1) * P, :])
                    nc.vector.tensor_add(x1[it][:], xr[:], ps[:])

            # ---- phase 7: LN2 (reuses uT tiles) ----
            with nc.named_scope("ln2"):
                layernorm_to_uT(lambda it: x1[it], uT, ps_m2)

            # ---- phase 8+9: MLP, chunk-interleaved so MLP2 of chunk 0
            # overlaps MLP1 of chunk 1 ----
            def mlp1_chunk(nch):
                for fh in range(FH):
                    ps = ps_acc.tile([P, C], F32, tag="acc")
                    for k in range(KC):
                        nc.tensor.matmul(
                            ps[:],
                            w1_t[k][:, fh * P : (fh + 1) * P],
                            uT[:, k, nch * 512 : (nch + 1) * 512],
                            start=(k == 0), stop=(k == KC - 1),
                        )
                    nc.scalar.activation(
                        out=h1T[fh][:, nch * 512 : (nch + 1) * 512], in_=ps[:],
                        func=mybir.ActivationFunctionType.Gelu,
                        bias=m1b_t[:, fh : fh + 1], scale=1.0,
                    )

            def mlp2_tiles(its):
                for it in its:
                    ps = ps_acc.tile([P, C], F32, tag="acc")
                    for fh in range(FH):
                        nc.tensor.matmul(
                            ps[:],
                            h1T[fh][:, it * P : (it + 1) * P],
                            w2_t[fh][:],
                            start=(fh == 0),
                            stop=(fh == FH - 1) and not with_biases,
                        )
                    if with_biases:
                        nc.tensor.matmul(
                            ps[:], ones_row[:], m2b_r[:], start=False, stop=True
                        )
                    ot = out_pool.tile([P, C], F32, tag="ot")
                    nc.vector.tensor_add(ot[:], x1[it][:], ps[:])
                    nc.sync.dma_start(out=y_d[it * P : (it + 1) * P, :], in_=ot[:])

            with nc.named_scope("mlp"):
                mlp1_chunk(0)
                mlp2_tiles(range(0, 4))
                mlp1_chunk(1)
                mlp2_tiles(range(4, 8))

    _split_excess_waits(nc)
    return nc


_PROGRAMS = {}


def _get_program(with_biases: bool):
    if with_biases not in _PROGRAMS:
        _PROGRAMS[with_biases] = _build_program(with_biases)
    return _PROGRAMS[with_biases]


def _gelu_exact(x):
    from math import sqrt

    from numpy import vectorize  # noqa: F401  (erf via scipy-free path below)

    # exact erf-based gelu in float64
    import numpy as _np

    # erf via numpy's math: use vectorized math.erf for accuracy
    from math import erf

    ex = _np.vectorize(erf)(x / sqrt(2.0))
    return 0.5 * x * (1.0 + ex)


def _reference_np(x_token, wq, wk, wv, w_proj, b_proj, g1, b1, g2, b2,
                  w1, bb1, w2, bb2, idx):
    """float64 numpy fallback (used only if the banded-kernel preconditions
    fail, e.g. a cluster larger than MAX_CLUSTER tokens)."""
    x = x_token.astype(np.float64)
    out = np.empty_like(x)
    scale = HD ** -0.5
    for b in range(x.shape[0]):
        xb = x[b]
        mu = xb.mean(-1, keepdims=True)
        var = ((xb - mu) ** 2).mean(-1, keepdims=True)
        t = (xb - mu) / np.sqrt(var + LN_EPS) * g1 + b1
        q = (t @ wq.T).reshape(N, H, HD).transpose(1, 0, 2)
        k = (t @ wk.T).reshape(N, H, HD).transpose(1, 0, 2)
        v = (t @ wv.T).reshape(N, H, HD).transpose(1, 0, 2)
        s = np.einsum("hid,hjd->hij", q, k) * scale
        same = idx[b][None, :, None] == idx[b][None, None, :]
        e = np.exp(s) * same
        attn = (e + ATT_EPS / N) / (e.sum(-1, keepdims=True) + ATT_EPS)
        o = np.einsum("hij,hjd->hid", attn, v)
        o = o.transpose(1, 0, 2).reshape(N, C) @ w_proj.T + b_proj
        xr = xb + o
        mu = xr.mean(-1, keepdims=True)
        var = ((xr - mu) ** 2).mean(-1, keepdims=True)
        hh = (xr - mu) / np.sqrt(var + LN_EPS) * g2 + b2
        m = _gelu_exact(hh @ w1.T + bb1) @ w2.T + bb2
        out[b] = xr + m
    return out.astype(np.float32)


def kernel(**inputs):
    x_token = np.ascontiguousarray(np.asarray(inputs["x_token"], np.float32))
    idx = np.asarray(inputs["idx_cluster"]).astype(np.int64)
    wq = np.asarray(inputs["wq"], np.float32)
    wk = np.asarray(inputs["wk"], np.float32)
    wv = np.asarray(inputs["wv"], np.float32)
    w_proj = np.asarray(inputs["w_proj"], np.float32)
    b_proj = np.asarray(inputs["b_proj"], np.float32)
    g1 = np.asarray(inputs["g1"], np.float32)
    b1 = np.asarray(inputs["b1"], np.float32)
    g2 = np.asarray(inputs["g2"], np.float32)
    b2 = np.asarray(inputs["b2"], np.float32)
    w1 = np.asarray(inputs["w1"], np.float32)
    bb1 = np.asarray(inputs["bb1"], np.float32)
    w2 = np.asarray(inputs["w2"], np.float32)
    bb2 = np.asarray(inputs["bb2"], np.float32)

    perms = []
    ok = idx.min() >= 0 and idx.max() < CLN
    if ok:
        for b in range(B):
            sizes = np.bincount(idx[b], minlength=CLN)
            if sizes.max() > MAX_CLUSTER:
                ok = False
                break
    if not ok:
        return _reference_np(x_token, wq, wk, wv, w_proj, b_proj, g1, b1,
                             g2, b2, w1, bb1, w2, bb2, idx)

    # fold LN gains/biases into adjacent weights
    wqT = (g1[:, None] * wq.T).astype(BF16)
    wkT = (g1[:, None] * wk.T).astype(BF16)
    wvT = (g1[:, None] * wv.T).astype(BF16)
    wpT = np.ascontiguousarray(w_proj.T).astype(BF16)
    w1T = (g2[:, None] * w1.T).astype(BF16)
    w2T = np.ascontiguousarray(w2.T).astype(BF16)
    qb = (wq @ b1).astype(np.float32)
    kb = (wk @ b1).astype(np.float32)
    vb = (wv @ b1).astype(BF16)
    pb = b_proj.astype(BF16)
    m1b = (bb1 + w1 @ b2).astype(np.float32)
    m2b = bb2.astype(BF16)

    # g1/g2/bb1/b2 fold into weights and the gelu bias (m1b), so any values
    # are handled by the fast path. Nonzero b1/b_proj/bb2 would need the
    # untested bias program variant -- setup_inputs() hardcodes them to
    # zero, so route that (unreachable) case to the numpy fallback instead.
    if np.any(qb) or np.any(kb) or np.any(vb) or np.any(pb) or np.any(m2b):
        return _reference_np(x_token, wq, wk, wv, w_proj, b_proj, g1, b1,
                             g2, b2, w1, bb1, w2, bb2, idx)
    with_biases = False

    shared = dict(
        wqT=wqT, wkT=wkT, wvT=wvT, wpT=wpT, w1T=w1T, w2T=w2T, m1b=m1b,
    )

    in_maps = []
    ar = np.arange(CLN)
    for b in range(B):
        perm = np.argsort(idx[b], kind="stable")
        perms.append(perm)
        cid = idx[b][perm]
        onehot = (cid[None, :] == ar[:, None])
        in_maps.append(
            dict(
                shared,
                x=np.ascontiguousarray(x_token[b][perm]),
                qmask=(ALPHA_Q * onehot).astype(BF16),
                kmask=(ALPHA_K * onehot).astype(BF16),
            )
        )

    nc = _get_program(with_biases)
    res = run_bass_kernel_spmd(nc, in_maps, list(range(B)))
    global LAST_RESULTS
    LAST_RESULTS = res
    out = np.empty((B, N, C), np.float32)
    for b in range(B):
        out[b][perms[b]] = res.results[b]["y"]
    return out


LAST_RESULTS = None
